# revision 12
# baseline (speedup 1.0000x reference)
"""Trainium2 Bass kernel for nn_LocalModel (6-encoder local-attention transformer).

Sharding: data-parallel over batch - B=8 batch elements, one per NeuronCore.
Each core runs the full 6-layer encoder stack + final projection for its
batch element on-chip (all weights resident in SBUF as fp16), returning a
[6]-vector; the host gathers them into the [8, 6] output.

Attention uses the zero-masked-softmax identity: with out-of-window scores
set to 0 (not -inf), softmax over the full sequence S satisfies
    out_i = (sum_{w in win} (e^{s_iw} - 1) vt_w + sum_all vt_j) / den_i + bv
    den_i = sum_{w in win} (e^{s_iw} - 1) + S
where vt = x @ wv.T (v without bias; bv moves outside the division).
Scores are computed in 256-wide diagonal chunks (chunk kc holds keys
[kc*128,(kc+1)*128) x queries [kc*128-64, kc*128+192)), so the banded
region is tight and one shared band mask serves every chunk. The
"+sum_all vt" and "+S" terms enter each PSUM accumulation via a K=1 matmul
against an augmented V-total row. out_w is prefetched (fp16, token-major)
during the layers and fused into layer 6 as vector dot products.
"""
import sys
import numpy as np

sys.path.insert(0, "/opt/trn_rl_repo")

B, S, D = 8, 1024, 512
H, Dh, W = 8, 64, 64
HD = 2048           # ffn hidden
C = 6               # classes
ENC = 6
EPS = 1e-5
P = 128
KO = D // P         # 4
HC = HD // P        # 16
SCALE = Dh ** -0.5

_CACHE = {}
LAST_EXEC_NS = None
LAST_RESULTS = None
TRACE = False


def _build(affine: bool, vbias: bool, b2: bool):
    import concourse.bass as bass
    import concourse.tile as tile
    from concourse import bacc, mybir
    from concourse.masks import make_identity

    f32 = mybir.dt.float32
    f16 = mybir.dt.float16
    AF = mybir.ActivationFunctionType
    OP = mybir.AluOpType

    nc = bacc.Bacc()
    d = {}
    d['xT'] = nc.declare_dram_parameter("xT", [D, S], f16, isOutput=False)
    for w in ("wqT", "wkT", "wvT"):
        d[w] = nc.declare_dram_parameter(w, [D, D], f16, isOutput=False)
    for b_ in ("bq", "bk"):
        d[b_] = nc.declare_dram_parameter(b_, [D], f32, isOutput=False)
    if vbias:
        d['bv'] = nc.declare_dram_parameter("bv", [D], f32, isOutput=False)
    d['fc1T'] = nc.declare_dram_parameter("fc1T", [D, HD], f16, isOutput=False)
    d['fc1b'] = nc.declare_dram_parameter("fc1b", [HD], f32, isOutput=False)
    d['fc2T'] = nc.declare_dram_parameter("fc2T", [HD, D], f16, isOutput=False)
    if b2:
        d['fc2b'] = nc.declare_dram_parameter("fc2b", [D], f32, isOutput=False)
    d['mask'] = nc.declare_dram_parameter("mask", [P, 256], f16, isOutput=False)
    d['owT'] = nc.declare_dram_parameter("owT", [C, 8, P, D], f16, isOutput=False)
    if affine:
        d['lng'] = nc.declare_dram_parameter("lng", [D], f32, isOutput=False)
        d['lnb'] = nc.declare_dram_parameter("lnb", [D], f32, isOutput=False)
    out_d = nc.declare_dram_parameter("out", [1, C], f32, isOutput=True)

    def bcast_ap(dram_h, parts=P):
        a = dram_h[:]
        return bass.AP(tensor=a.tensor, offset=a.offset,
                       ap=[[0, parts]] + [list(x) for x in a.ap])

    def bc_free(ap2, n):
        """append a 0-step broadcast dim of extent n to an AP"""
        return bass.AP(tensor=ap2.tensor, offset=ap2.offset,
                       ap=[list(x) for x in ap2.ap] + [[0, n]])

    def bc_mid(ap2, n):
        """[P, m] AP -> [P, n(0-step), m]"""
        a = [list(x) for x in ap2.ap]
        return bass.AP(tensor=ap2.tensor, offset=ap2.offset,
                       ap=[a[0], [0, n]] + a[1:])

    from contextlib import ExitStack
    with tile.TileContext(nc) as tc, ExitStack() as ctx:
        wpool = ctx.enter_context(tc.tile_pool(name="wpool", bufs=1))
        big = ctx.enter_context(tc.tile_pool(name="big", bufs=2))
        qkp = ctx.enter_context(tc.tile_pool(name="qkp", bufs=1))
        vap = ctx.enter_context(tc.tile_pool(name="vap", bufs=1))
        ppp = ctx.enter_context(tc.tile_pool(name="ppp", bufs=2))
        atp = ctx.enter_context(tc.tile_pool(name="atp", bufs=1))
        hp = ctx.enter_context(tc.tile_pool(name="hp", bufs=1))
        xnp = ctx.enter_context(tc.tile_pool(name="xnp", bufs=1))
        tmp = ctx.enter_context(tc.tile_pool(name="tmp", bufs=2))
        small = ctx.enter_context(tc.tile_pool(name="small", bufs=4))
        psQ = ctx.enter_context(tc.tile_pool(name="psQ", bufs=2, space="PSUM"))
        psS = ctx.enter_context(tc.tile_pool(name="psS", bufs=3, space="PSUM"))
        psV = ctx.enter_context(tc.tile_pool(name="psV", bufs=2, space="PSUM"))
        psT = ctx.enter_context(tc.tile_pool(name="psT", bufs=1, space="PSUM"))

        # ---- persistent loads (ordered roughly by first use) ----
        xT = big.tile([P, KO, S], f16, tag="big")
        nc.sync.dma_start(xT, d['xT'].rearrange("(ko p) n -> p ko n", p=P))
        wv_sb = wpool.tile([P, KO, D], f16, tag="wv")
        nc.sync.dma_start(wv_sb, d['wvT'].rearrange("(ko p) n -> p ko n", p=P))
        wq_sb = wpool.tile([P, KO, D], f16, tag="wq")
        nc.sync.dma_start(wq_sb, d['wqT'].rearrange("(ko p) n -> p ko n", p=P))
        wk_sb = wpool.tile([P, KO, D], f16, tag="wk")
        nc.sync.dma_start(wk_sb, d['wkT'].rearrange("(ko p) n -> p ko n", p=P))
        bq_sb = wpool.tile([P, KO], f32, tag="bq")
        bk_sb = wpool.tile([P, KO], f32, tag="bk")
        nc.sync.dma_start(bq_sb, d['bq'].rearrange("(ko p) -> p ko", p=P))
        nc.sync.dma_start(bk_sb, d['bk'].rearrange("(ko p) -> p ko", p=P))
        mask_sb = wpool.tile([P, 256], f16, tag="mask")
        nc.sync.dma_start(mask_sb, d['mask'][:])
        fc1_sb = wpool.tile([P, KO, HD], f16, tag="fc1")
        nc.sync.dma_start(fc1_sb, d['fc1T'].rearrange("(ko p) n -> p ko n", p=P))
        fc1b_sb = wpool.tile([P, HC], f32, tag="fc1b")
        nc.sync.dma_start(fc1b_sb, d['fc1b'].rearrange("(hc p) -> p hc", p=P))
        fc2_sb = wpool.tile([P, HC, D], f16, tag="fc2")
        nc.sync.dma_start(fc2_sb, d['fc2T'].rearrange("(hc p) n -> p hc n", p=P))
        if b2:
            fc2b_bc = wpool.tile([P, D], f32, tag="fc2b")
            nc.gpsimd.dma_start(out=fc2b_bc, in_=bcast_ap(d['fc2b']))
        if vbias:
            bv_bc = wpool.tile([P, D], f32, tag="bv")
            nc.gpsimd.dma_start(out=bv_bc, in_=bcast_ap(d['bv']))
        if affine:
            g_bc = wpool.tile([P, D], f32, tag="g")
            b_bc = wpool.tile([P, D], f32, tag="b")
            nc.gpsimd.dma_start(out=g_bc, in_=bcast_ap(d['lng']))
            nc.gpsimd.dma_start(out=b_bc, in_=bcast_ap(d['lnb']))
        ident = wpool.tile([P, P], f16, tag="id")
        make_identity(nc, ident)
        ones1p = wpool.tile([1, P], f16, tag="onr")
        nc.vector.memset(ones1p, 1.0)
        ones_col = wpool.tile([P, 1], f16, tag="onc")
        nc.vector.memset(ones_col, 1.0)
        eps_sb = wpool.tile([P, 1], f32, tag="eps")
        nc.vector.memset(eps_sb, EPS)
        # out_w prefetch (fp16, token-major) on the gpsimd queue
        owp = wpool.tile([P, C, 8, D], f16, tag="owp")
        for r in range(C):
            nc.gpsimd.dma_start(
                out=owp[:, r, :, :],
                in_=d['owT'][r, :, :, :].rearrange("t p f -> p t f"))

        vrow = wpool.tile([1, H * 65], f16, tag="vrow")
        red = wpool.tile([P, C * 8], f32, tag="red")
        mask_b = bc_mid(mask_sb[:, :], 8)  # [P, 8(bcast), 256]

        def layer_norm_to(src_ap, out_tile):
            """LayerNorm src [P,512] -> out_tile [P,512] fp16 (token-major)."""
            st = small.tile([P, 6], f32, tag="st")
            mv = small.tile([P, 2], f32, tag="mv")
            nc.vector.bn_stats(out=st, in_=src_ap)
            nc.vector.bn_aggr(out=mv, in_=st)
            rstd = small.tile([P, 1], f32, tag="rs")
            nc.scalar.activation(out=rstd, in_=mv[:, 1:2], func=AF.Sqrt,
                                 bias=eps_sb[:, 0:1])
            nc.vector.reciprocal(out=rstd, in_=rstd)
            nc.vector.tensor_scalar(out=out_tile, in0=src_ap,
                                    scalar1=mv[:, 0:1], scalar2=rstd,
                                    op0=OP.subtract, op1=OP.mult)
            if affine:
                nc.vector.tensor_tensor(out=out_tile, in0=out_tile, in1=g_bc,
                                        op=OP.mult)
                nc.vector.tensor_tensor(out=out_tile, in0=out_tile, in1=b_bc,
                                        op=OP.add)

        for L in range(ENC):
            last = (L == ENC - 1)
            # ---------- V totals row: vrow[h,0:64]=sum_t vt; vrow[h,64]=S ----
            xs32 = small.tile([P, KO], f32, tag="xs")
            nc.vector.reduce_sum(out=xs32, in_=xT, axis=mybir.AxisListType.X)
            xsr = small.tile([P, KO], f16, tag="xsr")
            nc.vector.tensor_copy(out=xsr, in_=xs32)
            pvt = psS.tile([1, 512], f32, tag="s")
            for ko in range(KO):
                nc.tensor.matmul(pvt, lhsT=xsr[:, ko:ko + 1],
                                 rhs=wv_sb[:, ko, :],
                                 start=(ko == 0), stop=(ko == KO - 1))
            nc.vector.tensor_copy(
                out=vrow.rearrange("q (h a) -> q h a", a=65)[:, :, 0:64],
                in_=pvt.rearrange("q (h a) -> q h a", a=64))
            nc.vector.memset(
                vrow.rearrange("q (h a) -> q h a", a=65)[:, :, 64:65],
                float(S))

            # ---------- interleaved projections + scores ---------------------
            # Per hko group: Q/K chunk, 2 V token-blocks, then the two heads'
            # scores issued pairwise (row groups 0-63 / 64-127 run
            # concurrently in the PE array). Dense N=512 projections stay
            # interleaved with the half-array score matmuls to keep HAM's
            # activity monitor above the throttle threshold.
            va = [None] * 8
            q_t, k_t = [], []
            pps = []
            for hko in range(KO):
                mc = hko
                qm = qkp.tile([P, S], f16, tag=f"q{mc}")
                km = qkp.tile([P, S], f16, tag=f"k{mc}")
                q_t.append(qm)
                k_t.append(km)
                for half in range(2):
                    cs = slice(half * 512, (half + 1) * 512)
                    pq = psQ.tile([P, 512], f32, tag="pj")
                    for ko in range(KO):
                        nc.tensor.matmul(
                            pq, lhsT=wq_sb[:, ko, mc * P:(mc + 1) * P],
                            rhs=xT[:, ko, cs],
                            start=(ko == 0), stop=(ko == KO - 1))
                    nc.scalar.activation(out=qm[:, cs], in_=pq,
                                         func=AF.Identity, bias=bq_sb[:, mc:mc + 1])
                    pk = psQ.tile([P, 512], f32, tag="pj")
                    for ko in range(KO):
                        nc.tensor.matmul(
                            pk, lhsT=wk_sb[:, ko, mc * P:(mc + 1) * P],
                            rhs=xT[:, ko, cs],
                            start=(ko == 0), stop=(ko == KO - 1))
                    nc.scalar.activation(out=km[:, cs], in_=pk,
                                         func=AF.Identity, bias=bk_sb[:, mc:mc + 1])
                # two V token-blocks (dense, keeps PE activity high)
                for tb in (2 * hko, 2 * hko + 1):
                    pv = psQ.tile([P, 512], f32, tag="pj")
                    for ko in range(KO):
                        nc.tensor.matmul(
                            pv, lhsT=xT[:, ko, tb * P:(tb + 1) * P],
                            rhs=wv_sb[:, ko, :],
                            start=(ko == 0), stop=(ko == KO - 1))
                    vt = vap.tile([P, H, 65], f16, tag=f"va{tb}")
                    nc.scalar.activation(
                        out=vt[:, :, 0:64],
                        in_=pv.rearrange("p (h a) -> p h a", a=64),
                        func=AF.Identity)
                    nc.vector.memset(vt[:, :, 64:65], 1.0)
                    va[tb] = vt
                # scores for heads (2*hko, 2*hko+1), row-group paired
                he, ho = 2 * hko, 2 * hko + 1
                ppe = ppp.tile([P, 8, 256], f16, tag=f"pp{he % 4}")
                ppo = ppp.tile([P, 8, 256], f16, tag=f"pp{ho % 4}")
                pse_f = ppe.rearrange("p a b -> p (a b)")
                pso_f = ppo.rearrange("p a b -> p (a b)")
                for pr in range(4):      # chunk pairs (2pr, 2pr+1) per bank
                    pse = psS.tile([P, 512], f32, tag="s")
                    pso = psS.tile([P, 512], f32, tag="s")
                    for half in range(2):
                        kc = 2 * pr + half
                        c0 = half * 256 + (64 if kc == 0 else 0)
                        c1 = half * 256 + 256 - (64 if kc == 7 else 0)
                        q0 = kc * P - 64 + (64 if kc == 0 else 0)
                        nc.tensor.matmul(
                            pse[:, c0:c1],
                            lhsT=k_t[hko][0:64, kc * P:(kc + 1) * P],
                            rhs=q_t[hko][0:64, q0:q0 + (c1 - c0)],
                            start=True, stop=True)
                        nc.tensor.matmul(
                            pso[:, c0:c1],
                            lhsT=k_t[hko][64:128, kc * P:(kc + 1) * P],
                            rhs=q_t[hko][64:128, q0:q0 + (c1 - c0)],
                            start=True, stop=True)
                    lo = 64 if pr == 0 else 0
                    hi = 448 if pr == 3 else 512
                    nc.scalar.activation(
                        out=pse_f[:, 512 * pr + lo:512 * pr + hi],
                        in_=pse[:, lo:hi], func=AF.Exp, scale=SCALE)
                    nc.scalar.activation(
                        out=pso_f[:, 512 * pr + lo:512 * pr + hi],
                        in_=pso[:, lo:hi], func=AF.Exp, scale=SCALE)
                for pp in (ppe, ppo):
                    nc.vector.scalar_tensor_tensor(
                        out=pp, in0=pp, scalar=1.0, in1=mask_b,
                        op0=OP.subtract, op1=OP.mult)
                pps.extend([ppe, ppo])

            # ---------- AV + epilogue, 4-head groups -------------------------
            a_tok = atp.tile([P, 8, D], f16, tag="at")
            x1T = big.tile([P, KO, S], f16, tag="big")
            xns = []
            for qb in range(8):
                for g in range(2):
                    pav = psV.tile([P, 260], f32, tag="av")
                    nc.tensor.matmul(pav, lhsT=ones1p,
                                     rhs=vrow[:, 260 * g:260 * (g + 1)],
                                     start=True, stop=False)
                    mms = []
                    for hh in range(4):
                        h = 4 * g + hh
                        cs = slice(65 * hh, 65 * hh + 65)
                        mms.append((pav[:, cs], pps[h][:, qb, 64:192], qb, h))
                        if qb > 0:
                            mms.append((pav[0:64, cs],
                                        pps[h][:, qb - 1, 192:256], qb - 1, h))
                        if qb < 7:
                            mms.append((pav[64:128, cs],
                                        pps[h][:, qb + 1, 0:64], qb + 1, h))
                    for i, (dst, lhsT, kc, h) in enumerate(mms):
                        nc.tensor.matmul(dst, lhsT=lhsT, rhs=va[kc][:, h, :],
                                         start=False, stop=(i == len(mms) - 1))
                    pavh = pav.rearrange("p (h a) -> p h a", a=65)
                    rc = small.tile([P, 4], f32, tag="rc")
                    nc.vector.reciprocal(
                        out=rc,
                        in_=pavh[:, :, 64:65].rearrange("p h a -> p (h a)"))
                    nc.vector.tensor_tensor(
                        out=a_tok[:, qb, 256 * g:256 * (g + 1)].rearrange(
                            "p (h a) -> p h a", a=64),
                        in0=pavh[:, :, 0:64],
                        in1=bc_free(rc[:, :], 64), op=OP.mult)
                # ---- LN1 + transpose for this block (overlaps later AV) ----
                if vbias:
                    nc.vector.tensor_tensor(out=a_tok[:, qb, :],
                                            in0=a_tok[:, qb, :], in1=bv_bc,
                                            op=OP.add)
                xn = xnp.tile([P, D], f16, tag=f"xn{qb}")
                layer_norm_to(a_tok[:, qb, :], xn)
                xns.append(xn)
                pt = psT.tile([P, KO, P], f16, tag="pt")
                for dc in range(KO):
                    nc.tensor.transpose(pt[:, dc, :],
                                        xn[:, dc * P:(dc + 1) * P], ident)
                nc.scalar.copy(out=x1T[:, :, qb * P:(qb + 1) * P], in_=pt)

            # ---------- FFN --------------------------------------------------
            if not last:
                xT_next = big.tile([P, KO, S], f16, tag="big")
            for th in range(2):
                ts = slice(th * 512, (th + 1) * 512)
                h_sb = hp.tile([P, HC, 512], f16, tag="h")
                for hc in range(HC):
                    ph = psQ.tile([P, 512], f32, tag="pj")
                    for ko in range(KO):
                        nc.tensor.matmul(
                            ph, lhsT=fc1_sb[:, ko, hc * P:(hc + 1) * P],
                            rhs=x1T[:, ko, ts],
                            start=(ko == 0), stop=(ko == KO - 1))
                    nc.scalar.activation(out=h_sb[:, hc, :], in_=ph,
                                         func=AF.Relu, bias=fc1b_sb[:, hc:hc + 1])
                for tw in range(4):
                    tb = th * 4 + tw
                    pf = psQ.tile([P, 512], f32, tag="pj")
                    for hc in range(HC):
                        nc.tensor.matmul(
                            pf, lhsT=h_sb[:, hc, tw * P:(tw + 1) * P],
                            rhs=fc2_sb[:, hc, :],
                            start=(hc == 0), stop=(hc == HC - 1))
                    f = tmp.tile([P, D], f16, tag="f")
                    if b2:
                        nc.vector.tensor_tensor(out=f, in0=pf, in1=fc2b_bc,
                                                op=OP.add)
                        nc.vector.tensor_tensor(out=f, in0=f, in1=xns[tb],
                                                op=OP.add)
                    else:
                        nc.vector.tensor_tensor(out=f, in0=pf, in1=xns[tb],
                                                op=OP.add)
                    xn2 = tmp.tile([P, D], f16, tag="x2")
                    layer_norm_to(f, xn2)
                    if not last:
                        pt = psT.tile([P, KO, P], f16, tag="pt")
                        for dc in range(KO):
                            nc.tensor.transpose(pt[:, dc, :],
                                                xn2[:, dc * P:(dc + 1) * P],
                                                ident)
                        nc.scalar.copy(out=xT_next[:, :, tb * P:(tb + 1) * P],
                                       in_=pt)
                    else:
                        junk = tmp.tile([P, D], f16, tag="jk")
                        for r in range(C):
                            nc.vector.scalar_tensor_tensor(
                                out=junk, in0=xn2, scalar=0.0, op0=OP.add,
                                in1=owp[:, r, tb, :], op1=OP.mult,
                                accum_out=red[:, r * 8 + tb:r * 8 + tb + 1])
            if not last:
                xT = xT_next

        # ---------- finish: out[r] = sum_p sum_tb red[p, r*8+tb] -------------
        red6 = wpool.tile([P, C], f32, tag="red6")
        nc.vector.reduce_sum(out=red6,
                             in_=red.rearrange("p (c t) -> p c t", t=8),
                             axis=mybir.AxisListType.X)
        red6h = wpool.tile([P, C], f16, tag="red6h")
        nc.vector.tensor_copy(out=red6h, in_=red6)
        pout = psS.tile([1, 512], f32, tag="s")
        nc.tensor.matmul(pout[0:1, 0:C], lhsT=ones_col[:, 0:1], rhs=red6h,
                         start=True, stop=True)
        osb = wpool.tile([1, C], f32, tag="osb")
        nc.scalar.copy(out=osb, in_=pout[0:1, 0:C])
        nc.sync.dma_start(out_d[:], osb)

    nc.compile()
    return nc


def _prep(inputs):
    """Host-side input prep shared across cores."""
    emb = np.asarray(inputs['emb'], dtype=np.float32)
    idx = np.asarray(inputs['inputs'])
    pos = np.arange(S, dtype=np.float32)[:, None]
    div = np.exp(-np.log(10000.0) * np.arange(0, D, 2, dtype=np.float32) / D)
    ang = pos * div
    pe = np.zeros((S, D), dtype=np.float32)
    pe[:, 0::2] = np.sin(ang)
    pe[:, 1::2] = np.cos(ang)
    x0 = emb[idx] + pe[None]  # [B, S, D]

    # band mask for one 256-wide diagonal chunk: valid iff 1 <= c - j <= 128
    jj = np.arange(P)[:, None]
    cc = np.arange(256)[None, :]
    mask = ((cc - jj >= 1) & (cc - jj <= 128)).astype(np.float16)

    ln_g = np.asarray(inputs['ln_g'], dtype=np.float32)
    ln_b = np.asarray(inputs['ln_b'], dtype=np.float32)
    affine = not (np.all(ln_g == 1.0) and np.all(ln_b == 0.0))
    bv = np.asarray(inputs['bv'], np.float32)
    vbias = bool(np.any(bv != 0.0))
    fc2b = np.asarray(inputs['fc2_b'], np.float32)
    b2 = bool(np.any(fc2b != 0.0))

    out_w = np.asarray(inputs['out_w'], dtype=np.float32)
    owT = np.ascontiguousarray(
        out_w.reshape(C, 8, P, D).astype(np.float16))

    common = {
        'wqT': np.ascontiguousarray(np.asarray(inputs['wq'], np.float32).T.astype(np.float16)),
        'wkT': np.ascontiguousarray(np.asarray(inputs['wk'], np.float32).T.astype(np.float16)),
        'wvT': np.ascontiguousarray(np.asarray(inputs['wv'], np.float32).T.astype(np.float16)),
        'bq': np.ascontiguousarray(np.asarray(inputs['bq'], np.float32)),
        'bk': np.ascontiguousarray(np.asarray(inputs['bk'], np.float32)),
        'fc1T': np.ascontiguousarray(np.asarray(inputs['fc1_w'], np.float32).T.astype(np.float16)),
        'fc1b': np.ascontiguousarray(np.asarray(inputs['fc1_b'], np.float32)),
        'fc2T': np.ascontiguousarray(np.asarray(inputs['fc2_w'], np.float32).T.astype(np.float16)),
        'mask': mask,
        'owT': owT,
    }
    if b2:
        common['fc2b'] = np.ascontiguousarray(fc2b)
    if vbias:
        common['bv'] = np.ascontiguousarray(bv)
    if affine:
        common['lng'] = np.ascontiguousarray(ln_g)
        common['lnb'] = np.ascontiguousarray(ln_b)
    per_core = [
        {'xT': np.ascontiguousarray(x0[b].T.astype(np.float16))}
        for b in range(B)
    ]
    return common, per_core, (affine, vbias, b2)


def kernel(**inputs):
    global LAST_EXEC_NS, LAST_RESULTS
    from concourse.bass_utils import run_bass_kernel_spmd

    common, per_core, flags = _prep(inputs)
    if flags not in _CACHE:
        _CACHE[flags] = _build(*flags)
    nc = _CACHE[flags]

    in_maps = [dict(common, **pc) for pc in per_core]
    res = run_bass_kernel_spmd(nc, in_maps, list(range(B)), trace=TRACE)
    LAST_EXEC_NS = res.exec_time_ns
    LAST_RESULTS = res
    out = np.stack([res.results[b]["out"][0] for b in range(B)], axis=0)
    out = out + np.asarray(inputs['out_b'], np.float32)[None, :]
    return out.astype(np.float32)


# revision 13
# speedup vs baseline: 1.0583x; 1.0583x over previous
"""Trainium2 Bass kernel for nn_LocalModel (6-encoder local-attention transformer).

Sharding: data-parallel over batch - B=8 batch elements, one per NeuronCore.
Each core runs the full 6-layer encoder stack + final projection for its
batch element on-chip (all weights resident in SBUF as fp16), returning a
[6]-vector; the host gathers them into the [8, 6] output.

Attention uses the zero-masked-softmax identity: with out-of-window scores
set to 0 (not -inf), softmax over the full sequence S satisfies
    out_i = (sum_{w in win} (e^{s_iw} - 1) vt_w + sum_all vt_j) / den_i + bv
    den_i = sum_{w in win} (e^{s_iw} - 1) + S
where vt = x @ wv.T (v without bias; bv moves outside the division).
Scores are computed in 256-wide diagonal chunks (chunk kc holds keys
[kc*128,(kc+1)*128) x queries [kc*128-64, kc*128+192)), so the banded
region is tight and one shared band mask serves every chunk. The
"+sum_all vt" and "+S" terms enter each PSUM accumulation via a K=1 matmul
against an augmented V-total row. out_w is prefetched (fp16, token-major)
during the layers and fused into layer 6 as vector dot products.
"""
import sys
import numpy as np

sys.path.insert(0, "/opt/trn_rl_repo")

B, S, D = 8, 1024, 512
H, Dh, W = 8, 64, 64
HD = 2048           # ffn hidden
C = 6               # classes
ENC = 6
EPS = 1e-5
P = 128
KO = D // P         # 4
HC = HD // P        # 16
SCALE = Dh ** -0.5

_CACHE = {}
LAST_EXEC_NS = None
LAST_RESULTS = None
TRACE = False


def _build(affine: bool, vbias: bool, b2: bool):
    import concourse.bass as bass
    import concourse.tile as tile
    from concourse import bacc, mybir
    from concourse.masks import make_identity

    f32 = mybir.dt.float32
    f16 = mybir.dt.float16
    AF = mybir.ActivationFunctionType
    OP = mybir.AluOpType

    nc = bacc.Bacc()
    d = {}
    d['xT'] = nc.declare_dram_parameter("xT", [D, S], f16, isOutput=False)
    for w in ("wqT", "wkT", "wvT"):
        d[w] = nc.declare_dram_parameter(w, [D, D], f16, isOutput=False)
    for b_ in ("bq", "bk"):
        d[b_] = nc.declare_dram_parameter(b_, [D], f32, isOutput=False)
    if vbias:
        d['bv'] = nc.declare_dram_parameter("bv", [D], f32, isOutput=False)
    d['fc1T'] = nc.declare_dram_parameter("fc1T", [D, HD], f16, isOutput=False)
    d['fc1b'] = nc.declare_dram_parameter("fc1b", [HD], f32, isOutput=False)
    d['fc2T'] = nc.declare_dram_parameter("fc2T", [HD, D], f16, isOutput=False)
    if b2:
        d['fc2b'] = nc.declare_dram_parameter("fc2b", [D], f32, isOutput=False)
    d['mask'] = nc.declare_dram_parameter("mask", [P, 256], f16, isOutput=False)
    d['owT'] = nc.declare_dram_parameter("owT", [C, 8, P, D], f16, isOutput=False)
    if affine:
        d['lng'] = nc.declare_dram_parameter("lng", [D], f32, isOutput=False)
        d['lnb'] = nc.declare_dram_parameter("lnb", [D], f32, isOutput=False)
    out_d = nc.declare_dram_parameter("out", [1, C], f32, isOutput=True)

    def bcast_ap(dram_h, parts=P):
        a = dram_h[:]
        return bass.AP(tensor=a.tensor, offset=a.offset,
                       ap=[[0, parts]] + [list(x) for x in a.ap])

    def bc_free(ap2, n):
        """append a 0-step broadcast dim of extent n to an AP"""
        return bass.AP(tensor=ap2.tensor, offset=ap2.offset,
                       ap=[list(x) for x in ap2.ap] + [[0, n]])

    def bc_mid(ap2, n):
        """[P, m] AP -> [P, n(0-step), m]"""
        a = [list(x) for x in ap2.ap]
        return bass.AP(tensor=ap2.tensor, offset=ap2.offset,
                       ap=[a[0], [0, n]] + a[1:])

    from contextlib import ExitStack
    with tile.TileContext(nc) as tc, ExitStack() as ctx:
        wpool = ctx.enter_context(tc.tile_pool(name="wpool", bufs=1))
        big = ctx.enter_context(tc.tile_pool(name="big", bufs=2))
        qkp = ctx.enter_context(tc.tile_pool(name="qkp", bufs=1))
        vap = ctx.enter_context(tc.tile_pool(name="vap", bufs=1))
        ppp = ctx.enter_context(tc.tile_pool(name="ppp", bufs=2))
        atp = ctx.enter_context(tc.tile_pool(name="atp", bufs=1))
        hp = ctx.enter_context(tc.tile_pool(name="hp", bufs=1))
        xnp = ctx.enter_context(tc.tile_pool(name="xnp", bufs=1))
        tmp = ctx.enter_context(tc.tile_pool(name="tmp", bufs=2))
        small = ctx.enter_context(tc.tile_pool(name="small", bufs=4))
        psQ = ctx.enter_context(tc.tile_pool(name="psQ", bufs=2, space="PSUM"))
        psS = ctx.enter_context(tc.tile_pool(name="psS", bufs=3, space="PSUM"))
        psV = ctx.enter_context(tc.tile_pool(name="psV", bufs=2, space="PSUM"))
        psT = ctx.enter_context(tc.tile_pool(name="psT", bufs=1, space="PSUM"))

        # ---- persistent loads (ordered roughly by first use) ----
        xT = big.tile([P, KO, S], f16, tag="big")
        nc.sync.dma_start(xT, d['xT'].rearrange("(ko p) n -> p ko n", p=P))
        wv_sb = wpool.tile([P, KO, D], f16, tag="wv")
        nc.sync.dma_start(wv_sb, d['wvT'].rearrange("(ko p) n -> p ko n", p=P))
        wq_sb = wpool.tile([P, KO, D], f16, tag="wq")
        nc.sync.dma_start(wq_sb, d['wqT'].rearrange("(ko p) n -> p ko n", p=P))
        wk_sb = wpool.tile([P, KO, D], f16, tag="wk")
        nc.sync.dma_start(wk_sb, d['wkT'].rearrange("(ko p) n -> p ko n", p=P))
        bq_sb = wpool.tile([P, KO], f32, tag="bq")
        bk_sb = wpool.tile([P, KO], f32, tag="bk")
        nc.sync.dma_start(bq_sb, d['bq'].rearrange("(ko p) -> p ko", p=P))
        nc.sync.dma_start(bk_sb, d['bk'].rearrange("(ko p) -> p ko", p=P))
        mask_sb = wpool.tile([P, 256], f16, tag="mask")
        nc.sync.dma_start(mask_sb, d['mask'][:])
        fc1_sb = wpool.tile([P, KO, HD], f16, tag="fc1")
        nc.sync.dma_start(fc1_sb, d['fc1T'].rearrange("(ko p) n -> p ko n", p=P))
        fc1b_sb = wpool.tile([P, HC], f32, tag="fc1b")
        nc.sync.dma_start(fc1b_sb, d['fc1b'].rearrange("(hc p) -> p hc", p=P))
        fc2_sb = wpool.tile([P, HC, D], f16, tag="fc2")
        nc.sync.dma_start(fc2_sb, d['fc2T'].rearrange("(hc p) n -> p hc n", p=P))
        if b2:
            fc2b_bc = wpool.tile([P, D], f32, tag="fc2b")
            nc.gpsimd.dma_start(out=fc2b_bc, in_=bcast_ap(d['fc2b']))
        if vbias:
            bv_bc = wpool.tile([P, D], f32, tag="bv")
            nc.gpsimd.dma_start(out=bv_bc, in_=bcast_ap(d['bv']))
        if affine:
            g_bc = wpool.tile([P, D], f32, tag="g")
            b_bc = wpool.tile([P, D], f32, tag="b")
            nc.gpsimd.dma_start(out=g_bc, in_=bcast_ap(d['lng']))
            nc.gpsimd.dma_start(out=b_bc, in_=bcast_ap(d['lnb']))
        ident = wpool.tile([P, P], f16, tag="id")
        make_identity(nc, ident)
        ones1p = wpool.tile([1, P], f16, tag="onr")
        nc.vector.memset(ones1p, 1.0)
        ones_col = wpool.tile([P, 1], f16, tag="onc")
        nc.vector.memset(ones_col, 1.0)
        eps_sb = wpool.tile([P, 1], f32, tag="eps")
        nc.vector.memset(eps_sb, EPS)
        # out_w prefetch (fp16, token-major) on the gpsimd queue
        owp = wpool.tile([P, C, 8, D], f16, tag="owp")
        for r in range(C):
            nc.gpsimd.dma_start(
                out=owp[:, r, :, :],
                in_=d['owT'][r, :, :, :].rearrange("t p f -> p t f"))

        vrow = wpool.tile([1, H * 65], f16, tag="vrow")
        red = wpool.tile([P, C * 8], f32, tag="red")
        mask_b = bc_mid(mask_sb[:, :], 8)  # [P, 8(bcast), 256]

        def layer_norm_to(src_ap, out_tile):
            """LayerNorm src [P,512] -> out_tile [P,512] fp16 (token-major)."""
            st = small.tile([P, 6], f32, tag="st")
            mv = small.tile([P, 2], f32, tag="mv")
            nc.vector.bn_stats(out=st, in_=src_ap)
            nc.vector.bn_aggr(out=mv, in_=st)
            rstd = small.tile([P, 1], f32, tag="rs")
            nc.scalar.activation(out=rstd, in_=mv[:, 1:2], func=AF.Sqrt,
                                 bias=eps_sb[:, 0:1])
            nc.vector.reciprocal(out=rstd, in_=rstd)
            nc.vector.tensor_scalar(out=out_tile, in0=src_ap,
                                    scalar1=mv[:, 0:1], scalar2=rstd,
                                    op0=OP.subtract, op1=OP.mult)
            if affine:
                nc.vector.tensor_tensor(out=out_tile, in0=out_tile, in1=g_bc,
                                        op=OP.mult)
                nc.vector.tensor_tensor(out=out_tile, in0=out_tile, in1=b_bc,
                                        op=OP.add)

        for L in range(ENC):
            last = (L == ENC - 1)
            # ---------- V totals row: vrow[h,0:64]=sum_t vt; vrow[h,64]=S ----
            xs32 = small.tile([P, KO], f32, tag="xs")
            nc.vector.reduce_sum(out=xs32, in_=xT, axis=mybir.AxisListType.X)
            xsr = small.tile([P, KO], f16, tag="xsr")
            nc.vector.tensor_copy(out=xsr, in_=xs32)
            pvt = psS.tile([1, 512], f32, tag="s")
            for ko in range(KO):
                nc.tensor.matmul(pvt, lhsT=xsr[:, ko:ko + 1],
                                 rhs=wv_sb[:, ko, :],
                                 start=(ko == 0), stop=(ko == KO - 1))
            nc.vector.tensor_copy(
                out=vrow.rearrange("q (h a) -> q h a", a=65)[:, :, 0:64],
                in_=pvt.rearrange("q (h a) -> q h a", a=64))
            nc.vector.memset(
                vrow.rearrange("q (h a) -> q h a", a=65)[:, :, 64:65],
                float(S))

            # ---------- interleaved projections + scores ---------------------
            # Per hko group: Q/K chunk, 2 V token-blocks, then the two heads'
            # scores issued pairwise (row groups 0-63 / 64-127 run
            # concurrently in the PE array). Dense N=512 projections stay
            # interleaved with the half-array score matmuls to keep HAM's
            # activity monitor above the throttle threshold.
            va = [None] * 8
            q_t, k_t = [], []
            pps = []
            for hko in range(KO):
                mc = hko
                qm = qkp.tile([P, S], f16, tag=f"q{mc}")
                km = qkp.tile([P, S], f16, tag=f"k{mc}")
                q_t.append(qm)
                k_t.append(km)
                for half in range(2):
                    cs = slice(half * 512, (half + 1) * 512)
                    pq = psQ.tile([P, 512], f32, tag="pj")
                    for ko in range(KO):
                        nc.tensor.matmul(
                            pq, lhsT=wq_sb[:, ko, mc * P:(mc + 1) * P],
                            rhs=xT[:, ko, cs],
                            start=(ko == 0), stop=(ko == KO - 1))
                    nc.scalar.activation(out=qm[:, cs], in_=pq,
                                         func=AF.Identity, bias=bq_sb[:, mc:mc + 1])
                    pk = psQ.tile([P, 512], f32, tag="pj")
                    for ko in range(KO):
                        nc.tensor.matmul(
                            pk, lhsT=wk_sb[:, ko, mc * P:(mc + 1) * P],
                            rhs=xT[:, ko, cs],
                            start=(ko == 0), stop=(ko == KO - 1))
                    nc.scalar.activation(out=km[:, cs], in_=pk,
                                         func=AF.Identity, bias=bk_sb[:, mc:mc + 1])
                # two V token-blocks (dense, keeps PE activity high)
                for tb in (2 * hko, 2 * hko + 1):
                    pv = psQ.tile([P, 512], f32, tag="pj")
                    for ko in range(KO):
                        nc.tensor.matmul(
                            pv, lhsT=xT[:, ko, tb * P:(tb + 1) * P],
                            rhs=wv_sb[:, ko, :],
                            start=(ko == 0), stop=(ko == KO - 1))
                    vt = vap.tile([P, H, 65], f16, tag=f"va{tb}")
                    nc.scalar.activation(
                        out=vt[:, :, 0:64],
                        in_=pv.rearrange("p (h a) -> p h a", a=64),
                        func=AF.Identity)
                    nc.vector.memset(vt[:, :, 64:65], 1.0)
                    va[tb] = vt
                # scores for heads (2*hko, 2*hko+1), row-group paired
                he, ho = 2 * hko, 2 * hko + 1
                ppe = ppp.tile([P, 8, 256], f16, tag=f"pp{he % 4}")
                ppo = ppp.tile([P, 8, 256], f16, tag=f"pp{ho % 4}")
                pse_f = ppe.rearrange("p a b -> p (a b)")
                pso_f = ppo.rearrange("p a b -> p (a b)")
                for pr in range(4):      # chunk pairs (2pr, 2pr+1) per bank
                    pse = psS.tile([P, 512], f32, tag="s")
                    pso = psS.tile([P, 512], f32, tag="s")
                    for half in range(2):
                        kc = 2 * pr + half
                        c0 = half * 256 + (64 if kc == 0 else 0)
                        c1 = half * 256 + 256 - (64 if kc == 7 else 0)
                        q0 = kc * P - 64 + (64 if kc == 0 else 0)
                        nc.tensor.matmul(
                            pse[:, c0:c1],
                            lhsT=k_t[hko][0:64, kc * P:(kc + 1) * P],
                            rhs=q_t[hko][0:64, q0:q0 + (c1 - c0)],
                            start=True, stop=True)
                        nc.tensor.matmul(
                            pso[:, c0:c1],
                            lhsT=k_t[hko][64:128, kc * P:(kc + 1) * P],
                            rhs=q_t[hko][64:128, q0:q0 + (c1 - c0)],
                            start=True, stop=True)
                    lo = 64 if pr == 0 else 0
                    hi = 448 if pr == 3 else 512
                    nc.scalar.activation(
                        out=pse_f[:, 512 * pr + lo:512 * pr + hi],
                        in_=pse[:, lo:hi], func=AF.Exp, scale=SCALE)
                    nc.scalar.activation(
                        out=pso_f[:, 512 * pr + lo:512 * pr + hi],
                        in_=pso[:, lo:hi], func=AF.Exp, scale=SCALE)
                for pp in (ppe, ppo):
                    nc.vector.scalar_tensor_tensor(
                        out=pp, in0=pp, scalar=1.0, in1=mask_b,
                        op0=OP.subtract, op1=OP.mult)
                pps.extend([ppe, ppo])

            # ---------- AV + epilogue, 4-head groups -------------------------
            a_tok = atp.tile([P, 8, D], f16, tag="at")
            x1T = big.tile([P, KO, S], f16, tag="big")
            xns = []
            for qb in range(8):
                for g in range(2):
                    pav = psV.tile([P, 260], f32, tag="av")
                    nc.tensor.matmul(pav, lhsT=ones1p,
                                     rhs=vrow[:, 260 * g:260 * (g + 1)],
                                     start=True, stop=False)
                    # mains first, then edges alternating col groups 0/64 so
                    # their LDWEIGHTS/compute overlap in disjoint quadrants
                    mms = []
                    for hh in range(4):
                        h = 4 * g + hh
                        cs = slice(65 * hh, 65 * hh + 65)
                        mms.append((pav[:, cs], pps[h][:, qb, 64:192], qb, h))
                    for hh in range(4):
                        h = 4 * g + hh
                        cs = slice(65 * hh, 65 * hh + 65)
                        if qb > 0:
                            mms.append((pav[0:64, cs],
                                        pps[h][:, qb - 1, 192:256], qb - 1, h))
                        if qb < 7:
                            mms.append((pav[64:128, cs],
                                        pps[h][:, qb + 1, 0:64], qb + 1, h))
                    for i, (dst, lhsT, kc, h) in enumerate(mms):
                        nc.tensor.matmul(dst, lhsT=lhsT, rhs=va[kc][:, h, :],
                                         start=False, stop=(i == len(mms) - 1))
                    pavh = pav.rearrange("p (h a) -> p h a", a=65)
                    rc = small.tile([P, 4], f32, tag="rc")
                    nc.vector.reciprocal(
                        out=rc,
                        in_=pavh[:, :, 64:65].rearrange("p h a -> p (h a)"))
                    nc.vector.tensor_tensor(
                        out=a_tok[:, qb, 256 * g:256 * (g + 1)].rearrange(
                            "p (h a) -> p h a", a=64),
                        in0=pavh[:, :, 0:64],
                        in1=bc_free(rc[:, :], 64), op=OP.mult)
                # ---- LN1 + transpose for this block (overlaps later AV) ----
                if vbias:
                    nc.vector.tensor_tensor(out=a_tok[:, qb, :],
                                            in0=a_tok[:, qb, :], in1=bv_bc,
                                            op=OP.add)
                xn = xnp.tile([P, D], f16, tag=f"xn{qb}")
                layer_norm_to(a_tok[:, qb, :], xn)
                xns.append(xn)
                pt = psT.tile([P, KO, P], f16, tag="pt")
                for dc in range(KO):
                    nc.tensor.transpose(pt[:, dc, :],
                                        xn[:, dc * P:(dc + 1) * P], ident)
                nc.scalar.copy(out=x1T[:, :, qb * P:(qb + 1) * P], in_=pt)

            # ---------- FFN --------------------------------------------------
            if not last:
                xT_next = big.tile([P, KO, S], f16, tag="big")
            for th in range(2):
                ts = slice(th * 512, (th + 1) * 512)
                h_sb = hp.tile([P, HC, 512], f16, tag="h")
                for hc in range(HC):
                    ph = psQ.tile([P, 512], f32, tag="pj")
                    for ko in range(KO):
                        nc.tensor.matmul(
                            ph, lhsT=fc1_sb[:, ko, hc * P:(hc + 1) * P],
                            rhs=x1T[:, ko, ts],
                            start=(ko == 0), stop=(ko == KO - 1))
                    nc.scalar.activation(out=h_sb[:, hc, :], in_=ph,
                                         func=AF.Relu, bias=fc1b_sb[:, hc:hc + 1])
                for tw in range(4):
                    tb = th * 4 + tw
                    pf = psQ.tile([P, 512], f32, tag="pj")
                    for hc in range(HC):
                        nc.tensor.matmul(
                            pf, lhsT=h_sb[:, hc, tw * P:(tw + 1) * P],
                            rhs=fc2_sb[:, hc, :],
                            start=(hc == 0), stop=(hc == HC - 1))
                    f = tmp.tile([P, D], f16, tag="f")
                    if b2:
                        nc.vector.tensor_tensor(out=f, in0=pf, in1=fc2b_bc,
                                                op=OP.add)
                        nc.vector.tensor_tensor(out=f, in0=f, in1=xns[tb],
                                                op=OP.add)
                    else:
                        nc.vector.tensor_tensor(out=f, in0=pf, in1=xns[tb],
                                                op=OP.add)
                    xn2 = tmp.tile([P, D], f16, tag="x2")
                    layer_norm_to(f, xn2)
                    if not last:
                        pt = psT.tile([P, KO, P], f16, tag="pt")
                        for dc in range(KO):
                            nc.tensor.transpose(pt[:, dc, :],
                                                xn2[:, dc * P:(dc + 1) * P],
                                                ident)
                        nc.scalar.copy(out=xT_next[:, :, tb * P:(tb + 1) * P],
                                       in_=pt)
                    else:
                        junk = tmp.tile([P, D], f16, tag="jk")
                        for r in range(C):
                            nc.vector.scalar_tensor_tensor(
                                out=junk, in0=xn2, scalar=0.0, op0=OP.add,
                                in1=owp[:, r, tb, :], op1=OP.mult,
                                accum_out=red[:, r * 8 + tb:r * 8 + tb + 1])
            if not last:
                xT = xT_next

        # ---------- finish: out[r] = sum_p sum_tb red[p, r*8+tb] -------------
        red6 = wpool.tile([P, C], f32, tag="red6")
        nc.vector.reduce_sum(out=red6,
                             in_=red.rearrange("p (c t) -> p c t", t=8),
                             axis=mybir.AxisListType.X)
        red6h = wpool.tile([P, C], f16, tag="red6h")
        nc.vector.tensor_copy(out=red6h, in_=red6)
        pout = psS.tile([1, 512], f32, tag="s")
        nc.tensor.matmul(pout[0:1, 0:C], lhsT=ones_col[:, 0:1], rhs=red6h,
                         start=True, stop=True)
        osb = wpool.tile([1, C], f32, tag="osb")
        nc.scalar.copy(out=osb, in_=pout[0:1, 0:C])
        nc.sync.dma_start(out_d[:], osb)

    nc.compile()
    return nc


def _prep(inputs):
    """Host-side input prep shared across cores."""
    emb = np.asarray(inputs['emb'], dtype=np.float32)
    idx = np.asarray(inputs['inputs'])
    pos = np.arange(S, dtype=np.float32)[:, None]
    div = np.exp(-np.log(10000.0) * np.arange(0, D, 2, dtype=np.float32) / D)
    ang = pos * div
    pe = np.zeros((S, D), dtype=np.float32)
    pe[:, 0::2] = np.sin(ang)
    pe[:, 1::2] = np.cos(ang)
    x0 = emb[idx] + pe[None]  # [B, S, D]

    # band mask for one 256-wide diagonal chunk: valid iff 1 <= c - j <= 128
    jj = np.arange(P)[:, None]
    cc = np.arange(256)[None, :]
    mask = ((cc - jj >= 1) & (cc - jj <= 128)).astype(np.float16)

    ln_g = np.asarray(inputs['ln_g'], dtype=np.float32)
    ln_b = np.asarray(inputs['ln_b'], dtype=np.float32)
    affine = not (np.all(ln_g == 1.0) and np.all(ln_b == 0.0))
    bv = np.asarray(inputs['bv'], np.float32)
    vbias = bool(np.any(bv != 0.0))
    fc2b = np.asarray(inputs['fc2_b'], np.float32)
    b2 = bool(np.any(fc2b != 0.0))

    out_w = np.asarray(inputs['out_w'], dtype=np.float32)
    owT = np.ascontiguousarray(
        out_w.reshape(C, 8, P, D).astype(np.float16))

    common = {
        'wqT': np.ascontiguousarray(np.asarray(inputs['wq'], np.float32).T.astype(np.float16)),
        'wkT': np.ascontiguousarray(np.asarray(inputs['wk'], np.float32).T.astype(np.float16)),
        'wvT': np.ascontiguousarray(np.asarray(inputs['wv'], np.float32).T.astype(np.float16)),
        'bq': np.ascontiguousarray(np.asarray(inputs['bq'], np.float32)),
        'bk': np.ascontiguousarray(np.asarray(inputs['bk'], np.float32)),
        'fc1T': np.ascontiguousarray(np.asarray(inputs['fc1_w'], np.float32).T.astype(np.float16)),
        'fc1b': np.ascontiguousarray(np.asarray(inputs['fc1_b'], np.float32)),
        'fc2T': np.ascontiguousarray(np.asarray(inputs['fc2_w'], np.float32).T.astype(np.float16)),
        'mask': mask,
        'owT': owT,
    }
    if b2:
        common['fc2b'] = np.ascontiguousarray(fc2b)
    if vbias:
        common['bv'] = np.ascontiguousarray(bv)
    if affine:
        common['lng'] = np.ascontiguousarray(ln_g)
        common['lnb'] = np.ascontiguousarray(ln_b)
    per_core = [
        {'xT': np.ascontiguousarray(x0[b].T.astype(np.float16))}
        for b in range(B)
    ]
    return common, per_core, (affine, vbias, b2)


def kernel(**inputs):
    global LAST_EXEC_NS, LAST_RESULTS
    from concourse.bass_utils import run_bass_kernel_spmd

    common, per_core, flags = _prep(inputs)
    if flags not in _CACHE:
        _CACHE[flags] = _build(*flags)
    nc = _CACHE[flags]

    in_maps = [dict(common, **pc) for pc in per_core]
    res = run_bass_kernel_spmd(nc, in_maps, list(range(B)), trace=TRACE)
    LAST_EXEC_NS = res.exec_time_ns
    LAST_RESULTS = res
    out = np.stack([res.results[b]["out"][0] for b in range(B)], axis=0)
    out = out + np.asarray(inputs['out_b'], np.float32)[None, :]
    return out.astype(np.float32)


# revision 15
# speedup vs baseline: 1.0760x; 1.0167x over previous
"""Trainium2 Bass kernel for nn_LocalModel (6-encoder local-attention transformer).

Sharding: data-parallel over batch - B=8 batch elements, one per NeuronCore.
Each core runs the full 6-layer encoder stack + final projection for its
batch element on-chip (all weights resident in SBUF as fp16), returning a
[6]-vector; the host gathers them into the [8, 6] output.

Attention uses the zero-masked-softmax identity: with out-of-window scores
set to 0 (not -inf), softmax over the full sequence S satisfies
    out_i = (sum_{w in win} (e^{s_iw} - 1) vt_w + sum_all vt_j) / den_i + bv
    den_i = sum_{w in win} (e^{s_iw} - 1) + S
where vt = x @ wv.T (v without bias; bv moves outside the division).
Scores are computed in 256-wide diagonal chunks (chunk kc holds keys
[kc*128,(kc+1)*128) x queries [kc*128-64, kc*128+192)), so the banded
region is tight and one shared band mask serves every chunk. The
"+sum_all vt" and "+S" terms enter each PSUM accumulation via a K=1 matmul
against an augmented V-total row. out_w is prefetched (fp16, token-major)
during the layers and fused into layer 6 as vector dot products.
"""
import sys
import numpy as np

sys.path.insert(0, "/opt/trn_rl_repo")

B, S, D = 8, 1024, 512
H, Dh, W = 8, 64, 64
HD = 2048           # ffn hidden
C = 6               # classes
ENC = 6
EPS = 1e-5
P = 128
KO = D // P         # 4
HC = HD // P        # 16
SCALE = Dh ** -0.5

_CACHE = {}
LAST_EXEC_NS = None
LAST_RESULTS = None
TRACE = False


def _build(affine: bool, vbias: bool, b2: bool):
    import concourse.bass as bass
    import concourse.tile as tile
    from concourse import bacc, mybir
    from concourse.masks import make_identity

    f32 = mybir.dt.float32
    f16 = mybir.dt.float16
    AF = mybir.ActivationFunctionType
    OP = mybir.AluOpType

    nc = bacc.Bacc()
    d = {}
    d['xT'] = nc.declare_dram_parameter("xT", [D, S], f16, isOutput=False)
    for w in ("wqT", "wkT", "wvT"):
        d[w] = nc.declare_dram_parameter(w, [D, D], f16, isOutput=False)
    for b_ in ("bq", "bk"):
        d[b_] = nc.declare_dram_parameter(b_, [D], f32, isOutput=False)
    if vbias:
        d['bv'] = nc.declare_dram_parameter("bv", [D], f32, isOutput=False)
    d['fc1T'] = nc.declare_dram_parameter("fc1T", [D, HD], f16, isOutput=False)
    d['fc1b'] = nc.declare_dram_parameter("fc1b", [HD], f32, isOutput=False)
    d['fc2T'] = nc.declare_dram_parameter("fc2T", [HD, D], f16, isOutput=False)
    if b2:
        d['fc2b'] = nc.declare_dram_parameter("fc2b", [D], f32, isOutput=False)
    d['mask'] = nc.declare_dram_parameter("mask", [P, 256], f16, isOutput=False)
    d['owT'] = nc.declare_dram_parameter("owT", [C, 8, P, D], f16, isOutput=False)
    if affine:
        d['lng'] = nc.declare_dram_parameter("lng", [D], f32, isOutput=False)
        d['lnb'] = nc.declare_dram_parameter("lnb", [D], f32, isOutput=False)
    out_d = nc.declare_dram_parameter("out", [1, C], f32, isOutput=True)

    def bcast_ap(dram_h, parts=P):
        a = dram_h[:]
        return bass.AP(tensor=a.tensor, offset=a.offset,
                       ap=[[0, parts]] + [list(x) for x in a.ap])

    def bc_free(ap2, n):
        """append a 0-step broadcast dim of extent n to an AP"""
        return bass.AP(tensor=ap2.tensor, offset=ap2.offset,
                       ap=[list(x) for x in ap2.ap] + [[0, n]])

    def bc_mid(ap2, n):
        """[P, m] AP -> [P, n(0-step), m]"""
        a = [list(x) for x in ap2.ap]
        return bass.AP(tensor=ap2.tensor, offset=ap2.offset,
                       ap=[a[0], [0, n]] + a[1:])

    from contextlib import ExitStack
    with tile.TileContext(nc) as tc, ExitStack() as ctx:
        wpool = ctx.enter_context(tc.tile_pool(name="wpool", bufs=1))
        big = ctx.enter_context(tc.tile_pool(name="big", bufs=2))
        qkp = ctx.enter_context(tc.tile_pool(name="qkp", bufs=1))
        vap = ctx.enter_context(tc.tile_pool(name="vap", bufs=1))
        ppp = ctx.enter_context(tc.tile_pool(name="ppp", bufs=2))
        atp = ctx.enter_context(tc.tile_pool(name="atp", bufs=1))
        hp = ctx.enter_context(tc.tile_pool(name="hp", bufs=1))
        xnp = ctx.enter_context(tc.tile_pool(name="xnp", bufs=1))
        tmp = ctx.enter_context(tc.tile_pool(name="tmp", bufs=2))
        small = ctx.enter_context(tc.tile_pool(name="small", bufs=4))
        psQ = ctx.enter_context(tc.tile_pool(name="psQ", bufs=2, space="PSUM"))
        psS = ctx.enter_context(tc.tile_pool(name="psS", bufs=3, space="PSUM"))
        psV = ctx.enter_context(tc.tile_pool(name="psV", bufs=2, space="PSUM"))
        psT = ctx.enter_context(tc.tile_pool(name="psT", bufs=1, space="PSUM"))

        # ---- persistent loads; xT/weights split across two DMA queues so
        # the V projection's inputs (xT + wv) land first ----
        xT = big.tile([P, KO, S], f16, tag="big")
        nc.sync.dma_start(xT[:, 0:2, :],
                          d['xT'].rearrange("(ko p) n -> p ko n", p=P)[:, 0:2, :])
        nc.scalar.dma_start(xT[:, 2:4, :],
                            d['xT'].rearrange("(ko p) n -> p ko n", p=P)[:, 2:4, :])
        wv_sb = wpool.tile([P, KO, D], f16, tag="wv")
        nc.sync.dma_start(wv_sb, d['wvT'].rearrange("(ko p) n -> p ko n", p=P))
        wq_sb = wpool.tile([P, KO, D], f16, tag="wq")
        nc.scalar.dma_start(wq_sb, d['wqT'].rearrange("(ko p) n -> p ko n", p=P))
        wk_sb = wpool.tile([P, KO, D], f16, tag="wk")
        nc.scalar.dma_start(wk_sb, d['wkT'].rearrange("(ko p) n -> p ko n", p=P))
        bq_sb = wpool.tile([P, KO], f32, tag="bq")
        bk_sb = wpool.tile([P, KO], f32, tag="bk")
        nc.scalar.dma_start(bq_sb, d['bq'].rearrange("(ko p) -> p ko", p=P))
        nc.scalar.dma_start(bk_sb, d['bk'].rearrange("(ko p) -> p ko", p=P))
        mask_sb = wpool.tile([P, 256], f16, tag="mask")
        nc.scalar.dma_start(mask_sb, d['mask'][:])
        fc1_sb = wpool.tile([P, KO, HD], f16, tag="fc1")
        nc.sync.dma_start(fc1_sb, d['fc1T'].rearrange("(ko p) n -> p ko n", p=P))
        fc1b_sb = wpool.tile([P, HC], f32, tag="fc1b")
        nc.scalar.dma_start(fc1b_sb, d['fc1b'].rearrange("(hc p) -> p hc", p=P))
        fc2_sb = wpool.tile([P, HC, D], f16, tag="fc2")
        nc.sync.dma_start(fc2_sb, d['fc2T'].rearrange("(hc p) n -> p hc n", p=P))
        if b2:
            fc2b_bc = wpool.tile([P, D], f32, tag="fc2b")
            nc.gpsimd.dma_start(out=fc2b_bc, in_=bcast_ap(d['fc2b']))
        if vbias:
            bv_bc = wpool.tile([P, D], f32, tag="bv")
            nc.gpsimd.dma_start(out=bv_bc, in_=bcast_ap(d['bv']))
        if affine:
            g_bc = wpool.tile([P, D], f32, tag="g")
            b_bc = wpool.tile([P, D], f32, tag="b")
            nc.gpsimd.dma_start(out=g_bc, in_=bcast_ap(d['lng']))
            nc.gpsimd.dma_start(out=b_bc, in_=bcast_ap(d['lnb']))
        ident = wpool.tile([P, P], f16, tag="id")
        make_identity(nc, ident)
        ones1p = wpool.tile([1, P], f16, tag="onr")
        nc.vector.memset(ones1p, 1.0)
        ones_col = wpool.tile([P, 1], f16, tag="onc")
        nc.vector.memset(ones_col, 1.0)
        eps_sb = wpool.tile([P, 1], f32, tag="eps")
        nc.vector.memset(eps_sb, EPS)
        # out_w prefetch (fp16, token-major) on the gpsimd queue
        owp = wpool.tile([P, C, 8, D], f16, tag="owp")
        for r in range(C):
            nc.gpsimd.dma_start(
                out=owp[:, r, :, :],
                in_=d['owT'][r, :, :, :].rearrange("t p f -> p t f"))

        vrow = wpool.tile([1, H * 65], f16, tag="vrow")
        red = wpool.tile([P, C * 8], f32, tag="red")
        mask_b = bc_mid(mask_sb[:, :], 8)  # [P, 8(bcast), 256]

        def layer_norm_to(src_ap, out_tile):
            """LayerNorm src [P,512] -> out_tile [P,512] fp16 (token-major)."""
            st = small.tile([P, 6], f32, tag="st")
            mv = small.tile([P, 2], f32, tag="mv")
            nc.vector.bn_stats(out=st, in_=src_ap)
            nc.vector.bn_aggr(out=mv, in_=st)
            rstd = small.tile([P, 1], f32, tag="rs")
            nc.scalar.activation(out=rstd, in_=mv[:, 1:2], func=AF.Sqrt,
                                 bias=eps_sb[:, 0:1])
            nc.vector.reciprocal(out=rstd, in_=rstd)
            nc.vector.tensor_scalar(out=out_tile, in0=src_ap,
                                    scalar1=mv[:, 0:1], scalar2=rstd,
                                    op0=OP.subtract, op1=OP.mult)
            if affine:
                nc.vector.tensor_tensor(out=out_tile, in0=out_tile, in1=g_bc,
                                        op=OP.mult)
                nc.vector.tensor_tensor(out=out_tile, in0=out_tile, in1=b_bc,
                                        op=OP.add)

        for L in range(ENC):
            last = (L == ENC - 1)
            # ---------- V totals row: vrow[h,0:64]=sum_t vt; vrow[h,64]=S ----
            xs32 = small.tile([P, KO], f32, tag="xs")
            nc.vector.reduce_sum(out=xs32, in_=xT, axis=mybir.AxisListType.X)
            xsr = small.tile([P, KO], f16, tag="xsr")
            nc.vector.tensor_copy(out=xsr, in_=xs32)
            pvt = psS.tile([1, 512], f32, tag="s")
            for ko in range(KO):
                nc.tensor.matmul(pvt, lhsT=xsr[:, ko:ko + 1],
                                 rhs=wv_sb[:, ko, :],
                                 start=(ko == 0), stop=(ko == KO - 1))
            nc.vector.tensor_copy(
                out=vrow.rearrange("q (h a) -> q h a", a=65)[:, :, 0:64],
                in_=pvt.rearrange("q (h a) -> q h a", a=64))
            nc.vector.memset(
                vrow.rearrange("q (h a) -> q h a", a=65)[:, :, 64:65],
                float(S))

            # ---------- interleaved projections + scores ---------------------
            # Per hko group: Q/K chunk, 2 V token-blocks, then the two heads'
            # scores issued pairwise (row groups 0-63 / 64-127 run
            # concurrently in the PE array). Dense N=512 projections stay
            # interleaved with the half-array score matmuls to keep HAM's
            # activity monitor above the throttle threshold.
            va = [None] * 8
            q_t, k_t = [], []
            pps = []
            for hko in range(KO):
                mc = hko
                qm = qkp.tile([P, S], f16, tag=f"q{mc}")
                km = qkp.tile([P, S], f16, tag=f"k{mc}")
                q_t.append(qm)
                k_t.append(km)
                for half in range(2):
                    cs = slice(half * 512, (half + 1) * 512)
                    pq = psQ.tile([P, 512], f32, tag="pj")
                    for ko in range(KO):
                        nc.tensor.matmul(
                            pq, lhsT=wq_sb[:, ko, mc * P:(mc + 1) * P],
                            rhs=xT[:, ko, cs],
                            start=(ko == 0), stop=(ko == KO - 1))
                    nc.scalar.activation(out=qm[:, cs], in_=pq,
                                         func=AF.Identity, bias=bq_sb[:, mc:mc + 1])
                    pk = psQ.tile([P, 512], f32, tag="pj")
                    for ko in range(KO):
                        nc.tensor.matmul(
                            pk, lhsT=wk_sb[:, ko, mc * P:(mc + 1) * P],
                            rhs=xT[:, ko, cs],
                            start=(ko == 0), stop=(ko == KO - 1))
                    nc.scalar.activation(out=km[:, cs], in_=pk,
                                         func=AF.Identity, bias=bk_sb[:, mc:mc + 1])
                # two V token-blocks (dense, keeps PE activity high)
                for tb in (2 * hko, 2 * hko + 1):
                    pv = psQ.tile([P, 512], f32, tag="pj")
                    for ko in range(KO):
                        nc.tensor.matmul(
                            pv, lhsT=xT[:, ko, tb * P:(tb + 1) * P],
                            rhs=wv_sb[:, ko, :],
                            start=(ko == 0), stop=(ko == KO - 1))
                    vt = vap.tile([P, H, 65], f16, tag=f"va{tb}")
                    nc.scalar.activation(
                        out=vt[:, :, 0:64],
                        in_=pv.rearrange("p (h a) -> p h a", a=64),
                        func=AF.Identity)
                    nc.vector.memset(vt[:, :, 64:65], 1.0)
                    va[tb] = vt
                # scores for heads (2*hko, 2*hko+1), row-group paired
                he, ho = 2 * hko, 2 * hko + 1
                ppe = ppp.tile([P, 8, 256], f16, tag=f"pp{he % 4}")
                ppo = ppp.tile([P, 8, 256], f16, tag=f"pp{ho % 4}")
                pse_f = ppe.rearrange("p a b -> p (a b)")
                pso_f = ppo.rearrange("p a b -> p (a b)")
                for pr in range(4):      # chunk pairs (2pr, 2pr+1) per bank
                    pse = psS.tile([P, 512], f32, tag="s")
                    pso = psS.tile([P, 512], f32, tag="s")
                    for half in range(2):
                        kc = 2 * pr + half
                        c0 = half * 256 + (64 if kc == 0 else 0)
                        c1 = half * 256 + 256 - (64 if kc == 7 else 0)
                        q0 = kc * P - 64 + (64 if kc == 0 else 0)
                        nc.tensor.matmul(
                            pse[:, c0:c1],
                            lhsT=k_t[hko][0:64, kc * P:(kc + 1) * P],
                            rhs=q_t[hko][0:64, q0:q0 + (c1 - c0)],
                            start=True, stop=True)
                        nc.tensor.matmul(
                            pso[:, c0:c1],
                            lhsT=k_t[hko][64:128, kc * P:(kc + 1) * P],
                            rhs=q_t[hko][64:128, q0:q0 + (c1 - c0)],
                            start=True, stop=True)
                    lo = 64 if pr == 0 else 0
                    hi = 448 if pr == 3 else 512
                    nc.scalar.activation(
                        out=pse_f[:, 512 * pr + lo:512 * pr + hi],
                        in_=pse[:, lo:hi], func=AF.Exp, scale=SCALE)
                    nc.scalar.activation(
                        out=pso_f[:, 512 * pr + lo:512 * pr + hi],
                        in_=pso[:, lo:hi], func=AF.Exp, scale=SCALE)
                for pp in (ppe, ppo):
                    nc.vector.scalar_tensor_tensor(
                        out=pp, in0=pp, scalar=1.0, in1=mask_b,
                        op0=OP.subtract, op1=OP.mult)
                pps.extend([ppe, ppo])

            # ---------- AV + epilogue, 4-head groups -------------------------
            a_tok = atp.tile([P, 8, D], f16, tag="at")
            x1T = big.tile([P, KO, S], f16, tag="big")
            xns = []
            for qb in range(8):
                for g in range(2):
                    pav = psV.tile([P, 260], f32, tag="av")
                    nc.tensor.matmul(pav, lhsT=ones1p,
                                     rhs=vrow[:, 260 * g:260 * (g + 1)],
                                     start=True, stop=False)
                    # all matmuls as M=64 halves alternating col groups 0/64
                    # so consecutive LDWEIGHTS/compute overlap in disjoint
                    # array quadrants (tile_position auto-derives from the
                    # output AP's base partition)
                    mms = []
                    for hh in range(4):
                        h = 4 * g + hh
                        cs = slice(65 * hh, 65 * hh + 65)
                        mms.append((pav[0:64, cs],
                                    pps[h][:, qb, 64:128], qb, h))
                        mms.append((pav[64:128, cs],
                                    pps[h][:, qb, 128:192], qb, h))
                    for hh in range(4):
                        h = 4 * g + hh
                        cs = slice(65 * hh, 65 * hh + 65)
                        if qb > 0:
                            mms.append((pav[0:64, cs],
                                        pps[h][:, qb - 1, 192:256], qb - 1, h))
                        if qb < 7:
                            mms.append((pav[64:128, cs],
                                        pps[h][:, qb + 1, 0:64], qb + 1, h))
                    for i, (dst, lhsT, kc, h) in enumerate(mms):
                        nc.tensor.matmul(dst, lhsT=lhsT, rhs=va[kc][:, h, :],
                                         start=False, stop=(i == len(mms) - 1))
                    pavh = pav.rearrange("p (h a) -> p h a", a=65)
                    rc = small.tile([P, 4], f32, tag="rc")
                    nc.vector.reciprocal(
                        out=rc,
                        in_=pavh[:, :, 64:65].rearrange("p h a -> p (h a)"))
                    nc.vector.tensor_tensor(
                        out=a_tok[:, qb, 256 * g:256 * (g + 1)].rearrange(
                            "p (h a) -> p h a", a=64),
                        in0=pavh[:, :, 0:64],
                        in1=bc_free(rc[:, :], 64), op=OP.mult)
                # ---- LN1 + transpose for this block (overlaps later AV) ----
                if vbias:
                    nc.vector.tensor_tensor(out=a_tok[:, qb, :],
                                            in0=a_tok[:, qb, :], in1=bv_bc,
                                            op=OP.add)
                xn = xnp.tile([P, D], f16, tag=f"xn{qb}")
                layer_norm_to(a_tok[:, qb, :], xn)
                xns.append(xn)
                pt = psT.tile([P, KO, P], f16, tag="pt")
                for dc in range(KO):
                    nc.tensor.transpose(pt[:, dc, :],
                                        xn[:, dc * P:(dc + 1) * P], ident)
                nc.scalar.copy(out=x1T[:, :, qb * P:(qb + 1) * P], in_=pt)

            # ---------- FFN --------------------------------------------------
            if not last:
                xT_next = big.tile([P, KO, S], f16, tag="big")
            for th in range(2):
                ts = slice(th * 512, (th + 1) * 512)
                h_sb = hp.tile([P, HC, 512], f16, tag="h")
                for hc in range(HC):
                    ph = psQ.tile([P, 512], f32, tag="pj")
                    for ko in range(KO):
                        nc.tensor.matmul(
                            ph, lhsT=fc1_sb[:, ko, hc * P:(hc + 1) * P],
                            rhs=x1T[:, ko, ts],
                            start=(ko == 0), stop=(ko == KO - 1))
                    nc.scalar.activation(out=h_sb[:, hc, :], in_=ph,
                                         func=AF.Relu, bias=fc1b_sb[:, hc:hc + 1])
                for tw in range(4):
                    tb = th * 4 + tw
                    pf = psQ.tile([P, 512], f32, tag="pj")
                    for hc in range(HC):
                        nc.tensor.matmul(
                            pf, lhsT=h_sb[:, hc, tw * P:(tw + 1) * P],
                            rhs=fc2_sb[:, hc, :],
                            start=(hc == 0), stop=(hc == HC - 1))
                    f = tmp.tile([P, D], f16, tag="f")
                    if b2:
                        nc.vector.tensor_tensor(out=f, in0=pf, in1=fc2b_bc,
                                                op=OP.add)
                        nc.vector.tensor_tensor(out=f, in0=f, in1=xns[tb],
                                                op=OP.add)
                    else:
                        nc.vector.tensor_tensor(out=f, in0=pf, in1=xns[tb],
                                                op=OP.add)
                    xn2 = tmp.tile([P, D], f16, tag="x2")
                    layer_norm_to(f, xn2)
                    if not last:
                        pt = psT.tile([P, KO, P], f16, tag="pt")
                        for dc in range(KO):
                            nc.tensor.transpose(pt[:, dc, :],
                                                xn2[:, dc * P:(dc + 1) * P],
                                                ident)
                        nc.scalar.copy(out=xT_next[:, :, tb * P:(tb + 1) * P],
                                       in_=pt)
                    else:
                        junk = tmp.tile([P, D], f16, tag="jk")
                        for r in range(C):
                            nc.vector.scalar_tensor_tensor(
                                out=junk, in0=xn2, scalar=0.0, op0=OP.add,
                                in1=owp[:, r, tb, :], op1=OP.mult,
                                accum_out=red[:, r * 8 + tb:r * 8 + tb + 1])
            if not last:
                xT = xT_next

        # ---------- finish: out[r] = sum_p sum_tb red[p, r*8+tb] -------------
        red6 = wpool.tile([P, C], f32, tag="red6")
        nc.vector.reduce_sum(out=red6,
                             in_=red.rearrange("p (c t) -> p c t", t=8),
                             axis=mybir.AxisListType.X)
        red6h = wpool.tile([P, C], f16, tag="red6h")
        nc.vector.tensor_copy(out=red6h, in_=red6)
        pout = psS.tile([1, 512], f32, tag="s")
        nc.tensor.matmul(pout[0:1, 0:C], lhsT=ones_col[:, 0:1], rhs=red6h,
                         start=True, stop=True)
        osb = wpool.tile([1, C], f32, tag="osb")
        nc.scalar.copy(out=osb, in_=pout[0:1, 0:C])
        nc.sync.dma_start(out_d[:], osb)

    nc.compile()
    return nc


def _prep(inputs):
    """Host-side input prep shared across cores."""
    emb = np.asarray(inputs['emb'], dtype=np.float32)
    idx = np.asarray(inputs['inputs'])
    pos = np.arange(S, dtype=np.float32)[:, None]
    div = np.exp(-np.log(10000.0) * np.arange(0, D, 2, dtype=np.float32) / D)
    ang = pos * div
    pe = np.zeros((S, D), dtype=np.float32)
    pe[:, 0::2] = np.sin(ang)
    pe[:, 1::2] = np.cos(ang)
    x0 = emb[idx] + pe[None]  # [B, S, D]

    # band mask for one 256-wide diagonal chunk: valid iff 1 <= c - j <= 128
    jj = np.arange(P)[:, None]
    cc = np.arange(256)[None, :]
    mask = ((cc - jj >= 1) & (cc - jj <= 128)).astype(np.float16)

    ln_g = np.asarray(inputs['ln_g'], dtype=np.float32)
    ln_b = np.asarray(inputs['ln_b'], dtype=np.float32)
    affine = not (np.all(ln_g == 1.0) and np.all(ln_b == 0.0))
    bv = np.asarray(inputs['bv'], np.float32)
    vbias = bool(np.any(bv != 0.0))
    fc2b = np.asarray(inputs['fc2_b'], np.float32)
    b2 = bool(np.any(fc2b != 0.0))

    out_w = np.asarray(inputs['out_w'], dtype=np.float32)
    owT = np.ascontiguousarray(
        out_w.reshape(C, 8, P, D).astype(np.float16))

    common = {
        'wqT': np.ascontiguousarray(np.asarray(inputs['wq'], np.float32).T.astype(np.float16)),
        'wkT': np.ascontiguousarray(np.asarray(inputs['wk'], np.float32).T.astype(np.float16)),
        'wvT': np.ascontiguousarray(np.asarray(inputs['wv'], np.float32).T.astype(np.float16)),
        'bq': np.ascontiguousarray(np.asarray(inputs['bq'], np.float32)),
        'bk': np.ascontiguousarray(np.asarray(inputs['bk'], np.float32)),
        'fc1T': np.ascontiguousarray(np.asarray(inputs['fc1_w'], np.float32).T.astype(np.float16)),
        'fc1b': np.ascontiguousarray(np.asarray(inputs['fc1_b'], np.float32)),
        'fc2T': np.ascontiguousarray(np.asarray(inputs['fc2_w'], np.float32).T.astype(np.float16)),
        'mask': mask,
        'owT': owT,
    }
    if b2:
        common['fc2b'] = np.ascontiguousarray(fc2b)
    if vbias:
        common['bv'] = np.ascontiguousarray(bv)
    if affine:
        common['lng'] = np.ascontiguousarray(ln_g)
        common['lnb'] = np.ascontiguousarray(ln_b)
    per_core = [
        {'xT': np.ascontiguousarray(x0[b].T.astype(np.float16))}
        for b in range(B)
    ]
    return common, per_core, (affine, vbias, b2)


def kernel(**inputs):
    global LAST_EXEC_NS, LAST_RESULTS
    from concourse.bass_utils import run_bass_kernel_spmd

    common, per_core, flags = _prep(inputs)
    if flags not in _CACHE:
        _CACHE[flags] = _build(*flags)
    nc = _CACHE[flags]

    in_maps = [dict(common, **pc) for pc in per_core]
    res = run_bass_kernel_spmd(nc, in_maps, list(range(B)), trace=TRACE)
    LAST_EXEC_NS = res.exec_time_ns
    LAST_RESULTS = res
    out = np.stack([res.results[b]["out"][0] for b in range(B)], axis=0)
    out = out + np.asarray(inputs['out_b'], np.float32)[None, :]
    return out.astype(np.float32)


# revision 20
# speedup vs baseline: 1.0888x; 1.0119x over previous
"""Trainium2 Bass kernel for nn_LocalModel (6-encoder local-attention transformer).

Sharding: data-parallel over batch - B=8 batch elements, one per NeuronCore.
Each core runs the full 6-layer encoder stack + final projection for its
batch element on-chip (all weights resident in SBUF as fp16), returning a
[6]-vector; the host gathers them into the [8, 6] output.

Attention uses the zero-masked-softmax identity: with out-of-window scores
set to 0 (not -inf), softmax over the full sequence S satisfies
    out_i = (sum_{w in win} (e^{s_iw} - 1) vt_w + sum_all vt_j) / den_i + bv
    den_i = sum_{w in win} (e^{s_iw} - 1) + S
where vt = x @ wv.T (v without bias; bv moves outside the division).
Scores are computed in 256-wide diagonal chunks (chunk kc holds keys
[kc*128,(kc+1)*128) x queries [kc*128-64, kc*128+192)), so the banded
region is tight and one shared band mask serves every chunk. The
"+sum_all vt" and "+S" terms enter each PSUM accumulation via a K=1 matmul
against an augmented V-total row. out_w is prefetched (fp16, token-major)
during the layers and fused into layer 6 as vector dot products.
"""
import sys
import numpy as np

sys.path.insert(0, "/opt/trn_rl_repo")

B, S, D = 8, 1024, 512
H, Dh, W = 8, 64, 64
HD = 2048           # ffn hidden
C = 6               # classes
ENC = 6
EPS = 1e-5
P = 128
KO = D // P         # 4
HC = HD // P        # 16
SCALE = Dh ** -0.5

_CACHE = {}
LAST_EXEC_NS = None
LAST_RESULTS = None
TRACE = False


def _build(affine: bool, vbias: bool, b2: bool):
    import concourse.bass as bass
    import concourse.tile as tile
    from concourse import bacc, mybir
    from concourse.masks import make_identity

    f32 = mybir.dt.float32
    f16 = mybir.dt.float16
    AF = mybir.ActivationFunctionType
    OP = mybir.AluOpType

    nc = bacc.Bacc()
    d = {}
    d['xT'] = nc.declare_dram_parameter("xT", [D, S], f16, isOutput=False)
    for w in ("wqT", "wkT", "wvT"):
        d[w] = nc.declare_dram_parameter(w, [D, D], f16, isOutput=False)
    for b_ in ("bq", "bk"):
        d[b_] = nc.declare_dram_parameter(b_, [D], f32, isOutput=False)
    if vbias:
        d['bv'] = nc.declare_dram_parameter("bv", [D], f32, isOutput=False)
    d['fc1T'] = nc.declare_dram_parameter("fc1T", [D, HD], f16, isOutput=False)
    d['fc1b'] = nc.declare_dram_parameter("fc1b", [HD], f32, isOutput=False)
    d['fc2T'] = nc.declare_dram_parameter("fc2T", [HD, D], f16, isOutput=False)
    if b2:
        d['fc2b'] = nc.declare_dram_parameter("fc2b", [D], f32, isOutput=False)
    d['mask'] = nc.declare_dram_parameter("mask", [P, 256], f16, isOutput=False)
    d['owT'] = nc.declare_dram_parameter("owT", [C, 8, P, D], f16, isOutput=False)
    if affine:
        d['lng'] = nc.declare_dram_parameter("lng", [D], f32, isOutput=False)
        d['lnb'] = nc.declare_dram_parameter("lnb", [D], f32, isOutput=False)
    out_d = nc.declare_dram_parameter("out", [1, C], f32, isOutput=True)

    def bcast_ap(dram_h, parts=P):
        a = dram_h[:]
        return bass.AP(tensor=a.tensor, offset=a.offset,
                       ap=[[0, parts]] + [list(x) for x in a.ap])

    def bc_free(ap2, n):
        """append a 0-step broadcast dim of extent n to an AP"""
        return bass.AP(tensor=ap2.tensor, offset=ap2.offset,
                       ap=[list(x) for x in ap2.ap] + [[0, n]])

    def bc_mid(ap2, n):
        """[P, m] AP -> [P, n(0-step), m]"""
        a = [list(x) for x in ap2.ap]
        return bass.AP(tensor=ap2.tensor, offset=ap2.offset,
                       ap=[a[0], [0, n]] + a[1:])

    from contextlib import ExitStack
    with tile.TileContext(nc) as tc, ExitStack() as ctx:
        wpool = ctx.enter_context(tc.tile_pool(name="wpool", bufs=1))
        big = ctx.enter_context(tc.tile_pool(name="big", bufs=2))
        qkp = ctx.enter_context(tc.tile_pool(name="qkp", bufs=1))
        vap = ctx.enter_context(tc.tile_pool(name="vap", bufs=1))
        ppp = ctx.enter_context(tc.tile_pool(name="ppp", bufs=2))
        atp = ctx.enter_context(tc.tile_pool(name="atp", bufs=1))
        hp = ctx.enter_context(tc.tile_pool(name="hp", bufs=1))
        xnp = ctx.enter_context(tc.tile_pool(name="xnp", bufs=1))
        tmp = ctx.enter_context(tc.tile_pool(name="tmp", bufs=2))
        small = ctx.enter_context(tc.tile_pool(name="small", bufs=4))
        psQ = ctx.enter_context(tc.tile_pool(name="psQ", bufs=2, space="PSUM"))
        psS = ctx.enter_context(tc.tile_pool(name="psS", bufs=3, space="PSUM"))
        psV = ctx.enter_context(tc.tile_pool(name="psV", bufs=2, space="PSUM"))
        psT = ctx.enter_context(tc.tile_pool(name="psT", bufs=1, space="PSUM"))

        # ---- persistent loads; spread across DMA queues so layer 1's
        # critical inputs (xT, wv, wq, wk) land as early as possible ----
        xT = big.tile([P, KO, S], f16, tag="big")
        xTd = d['xT'].rearrange("(ko p) n -> p ko n", p=P)
        nc.sync.dma_start(xT[:, 0:2, :], xTd[:, 0:2, :])
        nc.scalar.dma_start(xT[:, 2:4, :], xTd[:, 2:4, :])
        wv_sb = wpool.tile([P, KO, D], f16, tag="wv")
        nc.gpsimd.dma_start(out=wv_sb,
                            in_=d['wvT'].rearrange("(ko p) n -> p ko n", p=P))
        wq_sb = wpool.tile([P, KO, D], f16, tag="wq")
        nc.sync.dma_start(wq_sb, d['wqT'].rearrange("(ko p) n -> p ko n", p=P))
        wk_sb = wpool.tile([P, KO, D], f16, tag="wk")
        nc.scalar.dma_start(wk_sb, d['wkT'].rearrange("(ko p) n -> p ko n", p=P))
        bq_sb = wpool.tile([P, KO], f32, tag="bq")
        bk_sb = wpool.tile([P, KO], f32, tag="bk")
        nc.sync.dma_start(bq_sb, d['bq'].rearrange("(ko p) -> p ko", p=P))
        nc.scalar.dma_start(bk_sb, d['bk'].rearrange("(ko p) -> p ko", p=P))
        mask_sb = wpool.tile([P, 256], f16, tag="mask")
        nc.sync.dma_start(mask_sb, d['mask'][:])
        fc1_sb = wpool.tile([P, KO, HD], f16, tag="fc1")
        nc.sync.dma_start(fc1_sb, d['fc1T'].rearrange("(ko p) n -> p ko n", p=P))
        fc1b_sb = wpool.tile([P, HC], f32, tag="fc1b")
        nc.scalar.dma_start(fc1b_sb, d['fc1b'].rearrange("(hc p) -> p hc", p=P))
        fc2_sb = wpool.tile([P, HC, D], f16, tag="fc2")
        nc.scalar.dma_start(fc2_sb, d['fc2T'].rearrange("(hc p) n -> p hc n", p=P))
        if b2:
            fc2b_bc = wpool.tile([P, D], f32, tag="fc2b")
            nc.gpsimd.dma_start(out=fc2b_bc, in_=bcast_ap(d['fc2b']))
        if vbias:
            bv_bc = wpool.tile([P, D], f32, tag="bv")
            nc.gpsimd.dma_start(out=bv_bc, in_=bcast_ap(d['bv']))
        if affine:
            g_bc = wpool.tile([P, D], f32, tag="g")
            b_bc = wpool.tile([P, D], f32, tag="b")
            nc.gpsimd.dma_start(out=g_bc, in_=bcast_ap(d['lng']))
            nc.gpsimd.dma_start(out=b_bc, in_=bcast_ap(d['lnb']))
        ident = wpool.tile([P, P], f16, tag="id")
        make_identity(nc, ident)
        ones1p = wpool.tile([1, P], f16, tag="onr")
        nc.vector.memset(ones1p, 1.0)
        ones_col = wpool.tile([P, 1], f16, tag="onc")
        nc.vector.memset(ones_col, 1.0)
        eps_sb = wpool.tile([P, 1], f32, tag="eps")
        nc.vector.memset(eps_sb, EPS)
        # out_w prefetch (fp16, token-major) on the gpsimd queue
        owp = wpool.tile([P, C, 8, D], f16, tag="owp")
        for r in range(C):
            nc.gpsimd.dma_start(
                out=owp[:, r, :, :],
                in_=d['owT'][r, :, :, :].rearrange("t p f -> p t f"))

        vrow = wpool.tile([1, H * 65], f16, tag="vrow")
        red = wpool.tile([P, C * 8], f32, tag="red")
        mask_b = bc_mid(mask_sb[:, :], 8)  # [P, 8(bcast), 256]

        def layer_norm_to(src_ap, out_tile):
            """LayerNorm src [P,512] -> out_tile [P,512] fp16 (token-major)."""
            st = small.tile([P, 6], f32, tag="st")
            mv = small.tile([P, 2], f32, tag="mv")
            nc.vector.bn_stats(out=st, in_=src_ap)
            nc.vector.bn_aggr(out=mv, in_=st)
            rstd = small.tile([P, 1], f32, tag="rs")
            nc.scalar.activation(out=rstd, in_=mv[:, 1:2], func=AF.Sqrt,
                                 bias=eps_sb[:, 0:1])
            nc.vector.reciprocal(out=rstd, in_=rstd)
            nc.vector.tensor_scalar(out=out_tile, in0=src_ap,
                                    scalar1=mv[:, 0:1], scalar2=rstd,
                                    op0=OP.subtract, op1=OP.mult)
            if affine:
                nc.vector.tensor_tensor(out=out_tile, in0=out_tile, in1=g_bc,
                                        op=OP.mult)
                nc.vector.tensor_tensor(out=out_tile, in0=out_tile, in1=b_bc,
                                        op=OP.add)

        for L in range(ENC):
            last = (L == ENC - 1)
            # xsum for the V-totals row: issue the (slow) DVE reduce first so
            # it overlaps the whole projection phase; the dependent pvt
            # matmuls are issued AFTER the projection loop so the in-order PE
            # queue never head-of-line blocks on it.
            xs32 = small.tile([P, KO], f32, tag="xs")
            nc.vector.reduce_sum(out=xs32, in_=xT, axis=mybir.AxisListType.X)
            xsr = small.tile([P, KO], f16, tag="xsr")
            nc.vector.tensor_copy(out=xsr, in_=xs32)

            # ---------- interleaved projections + scores ---------------------
            # Per hko group: Q/K chunk, 2 V token-blocks, then the two heads'
            # scores issued pairwise (row groups 0-63 / 64-127 run
            # concurrently in the PE array). Dense N=512 projections stay
            # interleaved with the half-array score matmuls to keep HAM's
            # activity monitor above the throttle threshold.
            va = [None] * 8
            q_t, k_t = [], []
            pps = []
            for hko in range(KO):
                mc = hko
                qm = qkp.tile([P, S], f16, tag=f"q{mc}")
                km = qkp.tile([P, S], f16, tag=f"k{mc}")
                q_t.append(qm)
                k_t.append(km)
                for half in range(2):
                    cs = slice(half * 512, (half + 1) * 512)
                    pq = psQ.tile([P, 512], f32, tag="pj")
                    for ko in range(KO):
                        nc.tensor.matmul(
                            pq, lhsT=wq_sb[:, ko, mc * P:(mc + 1) * P],
                            rhs=xT[:, ko, cs],
                            start=(ko == 0), stop=(ko == KO - 1))
                    nc.scalar.activation(out=qm[:, cs], in_=pq,
                                         func=AF.Identity, bias=bq_sb[:, mc:mc + 1])
                    pk = psQ.tile([P, 512], f32, tag="pj")
                    for ko in range(KO):
                        nc.tensor.matmul(
                            pk, lhsT=wk_sb[:, ko, mc * P:(mc + 1) * P],
                            rhs=xT[:, ko, cs],
                            start=(ko == 0), stop=(ko == KO - 1))
                    nc.scalar.activation(out=km[:, cs], in_=pk,
                                         func=AF.Identity, bias=bk_sb[:, mc:mc + 1])
                # two V token-blocks (dense, keeps PE activity high)
                for tb in (2 * hko, 2 * hko + 1):
                    pv = psQ.tile([P, 512], f32, tag="pj")
                    for ko in range(KO):
                        nc.tensor.matmul(
                            pv, lhsT=xT[:, ko, tb * P:(tb + 1) * P],
                            rhs=wv_sb[:, ko, :],
                            start=(ko == 0), stop=(ko == KO - 1))
                    vt = vap.tile([P, H, 65], f16, tag=f"va{tb}")
                    nc.scalar.activation(
                        out=vt[:, :, 0:64],
                        in_=pv.rearrange("p (h a) -> p h a", a=64),
                        func=AF.Identity)
                    nc.vector.memset(vt[:, :, 64:65], 1.0)
                    va[tb] = vt
                # scores for heads (2*hko, 2*hko+1), row-group paired
                he, ho = 2 * hko, 2 * hko + 1
                ppe = ppp.tile([P, 8, 256], f16, tag=f"pp{he % 4}")
                ppo = ppp.tile([P, 8, 256], f16, tag=f"pp{ho % 4}")
                pse_f = ppe.rearrange("p a b -> p (a b)")
                pso_f = ppo.rearrange("p a b -> p (a b)")
                for pr in range(4):      # chunk pairs (2pr, 2pr+1) per bank
                    pse = psS.tile([P, 512], f32, tag="s")
                    pso = psS.tile([P, 512], f32, tag="s")
                    for half in range(2):
                        kc = 2 * pr + half
                        c0 = half * 256 + (64 if kc == 0 else 0)
                        c1 = half * 256 + 256 - (64 if kc == 7 else 0)
                        q0 = kc * P - 64 + (64 if kc == 0 else 0)
                        nc.tensor.matmul(
                            pse[:, c0:c1],
                            lhsT=k_t[hko][0:64, kc * P:(kc + 1) * P],
                            rhs=q_t[hko][0:64, q0:q0 + (c1 - c0)],
                            start=True, stop=True)
                        nc.tensor.matmul(
                            pso[:, c0:c1],
                            lhsT=k_t[hko][64:128, kc * P:(kc + 1) * P],
                            rhs=q_t[hko][64:128, q0:q0 + (c1 - c0)],
                            start=True, stop=True)
                    lo = 64 if pr == 0 else 0
                    hi = 448 if pr == 3 else 512
                    nc.scalar.activation(
                        out=pse_f[:, 512 * pr + lo:512 * pr + hi],
                        in_=pse[:, lo:hi], func=AF.Exp, scale=SCALE)
                    nc.scalar.activation(
                        out=pso_f[:, 512 * pr + lo:512 * pr + hi],
                        in_=pso[:, lo:hi], func=AF.Exp, scale=SCALE)
                for pp in (ppe, ppo):
                    nc.vector.scalar_tensor_tensor(
                        out=pp, in0=pp, scalar=1.0, in1=mask_b,
                        op0=OP.subtract, op1=OP.mult)
                pps.extend([ppe, ppo])

            # ---------- V totals row: vrow[h,0:64]=sum_t vt; vrow[h,64]=S ----
            pvt = psS.tile([1, 512], f32, tag="s")
            for ko in range(KO):
                nc.tensor.matmul(pvt, lhsT=xsr[:, ko:ko + 1],
                                 rhs=wv_sb[:, ko, :],
                                 start=(ko == 0), stop=(ko == KO - 1))
            nc.vector.tensor_copy(
                out=vrow.rearrange("q (h a) -> q h a", a=65)[:, :, 0:64],
                in_=pvt.rearrange("q (h a) -> q h a", a=64))
            nc.vector.memset(
                vrow.rearrange("q (h a) -> q h a", a=65)[:, :, 64:65],
                float(S))

            # ---------- AV + epilogue, 4-head groups -------------------------
            a_tok = atp.tile([P, 8, D], f16, tag="at")
            x1T = big.tile([P, KO, S], f16, tag="big")
            xns = []
            for qb in range(8):
                for g in range(2):
                    pav = psV.tile([P, 260], f32, tag="av")
                    nc.tensor.matmul(pav, lhsT=ones1p,
                                     rhs=vrow[:, 260 * g:260 * (g + 1)],
                                     start=True, stop=False)
                    # all matmuls as M=64 halves alternating col groups 0/64
                    # so consecutive LDWEIGHTS/compute overlap in disjoint
                    # array quadrants (tile_position auto-derives from the
                    # output AP's base partition)
                    mms = []
                    for hh in range(4):
                        h = 4 * g + hh
                        cs = slice(65 * hh, 65 * hh + 65)
                        mms.append((pav[0:64, cs],
                                    pps[h][:, qb, 64:128], qb, h))
                        mms.append((pav[64:128, cs],
                                    pps[h][:, qb, 128:192], qb, h))
                    for hh in range(4):
                        h = 4 * g + hh
                        cs = slice(65 * hh, 65 * hh + 65)
                        if qb > 0:
                            mms.append((pav[0:64, cs],
                                        pps[h][:, qb - 1, 192:256], qb - 1, h))
                        if qb < 7:
                            mms.append((pav[64:128, cs],
                                        pps[h][:, qb + 1, 0:64], qb + 1, h))
                    for i, (dst, lhsT, kc, h) in enumerate(mms):
                        nc.tensor.matmul(dst, lhsT=lhsT, rhs=va[kc][:, h, :],
                                         start=False, stop=(i == len(mms) - 1))
                    pavh = pav.rearrange("p (h a) -> p h a", a=65)
                    rc = small.tile([P, 4], f32, tag="rc")
                    nc.vector.reciprocal(
                        out=rc,
                        in_=pavh[:, :, 64:65].rearrange("p h a -> p (h a)"))
                    nc.vector.tensor_tensor(
                        out=a_tok[:, qb, 256 * g:256 * (g + 1)].rearrange(
                            "p (h a) -> p h a", a=64),
                        in0=pavh[:, :, 0:64],
                        in1=bc_free(rc[:, :], 64), op=OP.mult)
                # ---- LN1 + transpose for this block (overlaps later AV) ----
                if vbias:
                    nc.vector.tensor_tensor(out=a_tok[:, qb, :],
                                            in0=a_tok[:, qb, :], in1=bv_bc,
                                            op=OP.add)
                xn = xnp.tile([P, D], f16, tag=f"xn{qb}")
                layer_norm_to(a_tok[:, qb, :], xn)
                xns.append(xn)
                pt = psT.tile([P, KO, P], f16, tag="pt")
                for dc in range(KO):
                    nc.tensor.transpose(pt[:, dc, :],
                                        xn[:, dc * P:(dc + 1) * P], ident)
                nc.scalar.copy(out=x1T[:, :, qb * P:(qb + 1) * P], in_=pt)

            # ---------- FFN --------------------------------------------------
            if not last:
                xT_next = big.tile([P, KO, S], f16, tag="big")
            for th in range(2):
                ts = slice(th * 512, (th + 1) * 512)
                h_sb = hp.tile([P, HC, 512], f16, tag="h")
                for hc in range(HC):
                    ph = psQ.tile([P, 512], f32, tag="pj")
                    for ko in range(KO):
                        nc.tensor.matmul(
                            ph, lhsT=fc1_sb[:, ko, hc * P:(hc + 1) * P],
                            rhs=x1T[:, ko, ts],
                            start=(ko == 0), stop=(ko == KO - 1))
                    nc.scalar.activation(out=h_sb[:, hc, :], in_=ph,
                                         func=AF.Relu, bias=fc1b_sb[:, hc:hc + 1])
                for tw in range(4):
                    tb = th * 4 + tw
                    pf = psQ.tile([P, 512], f32, tag="pj")
                    for hc in range(HC):
                        nc.tensor.matmul(
                            pf, lhsT=h_sb[:, hc, tw * P:(tw + 1) * P],
                            rhs=fc2_sb[:, hc, :],
                            start=(hc == 0), stop=(hc == HC - 1))
                    f = tmp.tile([P, D], f16, tag="f")
                    if b2:
                        nc.vector.tensor_tensor(out=f, in0=pf, in1=fc2b_bc,
                                                op=OP.add)
                        nc.vector.tensor_tensor(out=f, in0=f, in1=xns[tb],
                                                op=OP.add)
                    else:
                        nc.vector.tensor_tensor(out=f, in0=pf, in1=xns[tb],
                                                op=OP.add)
                    xn2 = tmp.tile([P, D], f16, tag="x2")
                    layer_norm_to(f, xn2)
                    if not last:
                        pt = psT.tile([P, KO, P], f16, tag="pt")
                        for dc in range(KO):
                            nc.tensor.transpose(pt[:, dc, :],
                                                xn2[:, dc * P:(dc + 1) * P],
                                                ident)
                        nc.scalar.copy(out=xT_next[:, :, tb * P:(tb + 1) * P],
                                       in_=pt)
                    else:
                        junk = tmp.tile([P, D], f16, tag="jk")
                        for r in range(C):
                            nc.vector.scalar_tensor_tensor(
                                out=junk, in0=xn2, scalar=0.0, op0=OP.add,
                                in1=owp[:, r, tb, :], op1=OP.mult,
                                accum_out=red[:, r * 8 + tb:r * 8 + tb + 1])
            if not last:
                xT = xT_next

        # ---------- finish: out[r] = sum_p sum_tb red[p, r*8+tb] -------------
        red6 = wpool.tile([P, C], f32, tag="red6")
        nc.vector.reduce_sum(out=red6,
                             in_=red.rearrange("p (c t) -> p c t", t=8),
                             axis=mybir.AxisListType.X)
        red6h = wpool.tile([P, C], f16, tag="red6h")
        nc.vector.tensor_copy(out=red6h, in_=red6)
        pout = psS.tile([1, 512], f32, tag="s")
        nc.tensor.matmul(pout[0:1, 0:C], lhsT=ones_col[:, 0:1], rhs=red6h,
                         start=True, stop=True)
        osb = wpool.tile([1, C], f32, tag="osb")
        nc.scalar.copy(out=osb, in_=pout[0:1, 0:C])
        nc.sync.dma_start(out_d[:], osb)

    nc.compile()
    return nc


def _prep(inputs):
    """Host-side input prep shared across cores."""
    emb = np.asarray(inputs['emb'], dtype=np.float32)
    idx = np.asarray(inputs['inputs'])
    pos = np.arange(S, dtype=np.float32)[:, None]
    div = np.exp(-np.log(10000.0) * np.arange(0, D, 2, dtype=np.float32) / D)
    ang = pos * div
    pe = np.zeros((S, D), dtype=np.float32)
    pe[:, 0::2] = np.sin(ang)
    pe[:, 1::2] = np.cos(ang)
    x0 = emb[idx] + pe[None]  # [B, S, D]

    # band mask for one 256-wide diagonal chunk: valid iff 1 <= c - j <= 128
    jj = np.arange(P)[:, None]
    cc = np.arange(256)[None, :]
    mask = ((cc - jj >= 1) & (cc - jj <= 128)).astype(np.float16)

    ln_g = np.asarray(inputs['ln_g'], dtype=np.float32)
    ln_b = np.asarray(inputs['ln_b'], dtype=np.float32)
    affine = not (np.all(ln_g == 1.0) and np.all(ln_b == 0.0))
    bv = np.asarray(inputs['bv'], np.float32)
    vbias = bool(np.any(bv != 0.0))
    fc2b = np.asarray(inputs['fc2_b'], np.float32)
    b2 = bool(np.any(fc2b != 0.0))

    out_w = np.asarray(inputs['out_w'], dtype=np.float32)
    owT = np.ascontiguousarray(
        out_w.reshape(C, 8, P, D).astype(np.float16))

    common = {
        'wqT': np.ascontiguousarray(np.asarray(inputs['wq'], np.float32).T.astype(np.float16)),
        'wkT': np.ascontiguousarray(np.asarray(inputs['wk'], np.float32).T.astype(np.float16)),
        'wvT': np.ascontiguousarray(np.asarray(inputs['wv'], np.float32).T.astype(np.float16)),
        'bq': np.ascontiguousarray(np.asarray(inputs['bq'], np.float32)),
        'bk': np.ascontiguousarray(np.asarray(inputs['bk'], np.float32)),
        'fc1T': np.ascontiguousarray(np.asarray(inputs['fc1_w'], np.float32).T.astype(np.float16)),
        'fc1b': np.ascontiguousarray(np.asarray(inputs['fc1_b'], np.float32)),
        'fc2T': np.ascontiguousarray(np.asarray(inputs['fc2_w'], np.float32).T.astype(np.float16)),
        'mask': mask,
        'owT': owT,
    }
    if b2:
        common['fc2b'] = np.ascontiguousarray(fc2b)
    if vbias:
        common['bv'] = np.ascontiguousarray(bv)
    if affine:
        common['lng'] = np.ascontiguousarray(ln_g)
        common['lnb'] = np.ascontiguousarray(ln_b)
    per_core = [
        {'xT': np.ascontiguousarray(x0[b].T.astype(np.float16))}
        for b in range(B)
    ]
    return common, per_core, (affine, vbias, b2)


def kernel(**inputs):
    global LAST_EXEC_NS, LAST_RESULTS
    from concourse.bass_utils import run_bass_kernel_spmd

    common, per_core, flags = _prep(inputs)
    if flags not in _CACHE:
        _CACHE[flags] = _build(*flags)
    nc = _CACHE[flags]

    in_maps = [dict(common, **pc) for pc in per_core]
    res = run_bass_kernel_spmd(nc, in_maps, list(range(B)), trace=TRACE)
    LAST_EXEC_NS = res.exec_time_ns
    LAST_RESULTS = res
    out = np.stack([res.results[b]["out"][0] for b in range(B)], axis=0)
    out = out + np.asarray(inputs['out_b'], np.float32)[None, :]
    return out.astype(np.float32)
